# revision 2
# baseline (speedup 1.0000x reference)
"""Self-contained Trainium2 Bass kernel for GQA MultiHeadAttention with RoPE.

Problem: B=2, S=2048, D=1024, H=16 Q heads, KVH=4 KV heads, head_dim=64,
causal additive mask, f32 in/out.

Sharding: TP=4 over heads (4 Q heads + 1 KV head per shard) x DP=2 over
batch = 8 NeuronCores. Wo sharded on input dim; host sums the 4 partial
outputs per batch element.

Design notes (v2):
- All-bf16 datapath (weights, hidden, q/k/v, exp probs, ctx, out partials);
  psum accumulation stays f32. Error budget 2e-2 rel leaves ~7x headroom.
- Phase A (projections+rope) is emitted strip-wise (512 seq cols) and
  interleaved with attention blocks so PE never idles on exp latency.
- Softmax normalization is fused on-psum: DVE reciprocal of the rowsum row
  (row 64 of the ctx psum, produced by an ones-column in vsm), PE ones-matmul
  broadcast of the recip row, DVE multiply straight out of psum into ctxT.
  No DRAM scratch, no reshape bounces.
- Diagonal (causal-edge) tiles are batched into 2 grouped psum slots per
  (head, qb) so exps are [128, 896]+[128, 384] instead of 4 small ones.
- DMAs are coalesced (one per hT strip, one out-store per 128-row tile) and
  issued on the engine that produced their source.
"""

import os
import sys

for _p in ("/opt/trn_rl_repo", "/root/.axon_site/_ro/trn_rl_repo"):
    if os.path.isdir(_p) and _p not in sys.path:
        sys.path.insert(0, _p)

import numpy as np
import ml_dtypes

import concourse.bacc as bacc
import concourse.bass as bass
import concourse.tile as tile
from concourse import mybir
from concourse.bass_utils import run_bass_kernel_spmd

F32 = mybir.dt.float32
F32R = mybir.dt.float32r
BF16 = mybir.dt.bfloat16
AF = mybir.ActivationFunctionType

H, KVH, HD = 16, 4, 64
B, S, D = 2, 2048, 1024
TP = 4
SCALE = HD ** -0.5
NEG = -1e9
NT = S // 128               # 16 kv tiles
NQB = S // 512              # 4 q blocks


def _interleave(a, b):
    """Merge two unit lists proportionally (a is the primary stream)."""
    if not b:
        return list(a)
    if not a:
        return list(b)
    out = []
    na, nb = len(a), len(b)
    ia = ib = 0
    while ia < na or ib < nb:
        # emit whichever stream is behind proportionally
        if ib >= nb or (ia < na and ia * nb <= ib * na):
            out.append(a[ia]); ia += 1
        else:
            out.append(b[ib]); ib += 1
    return out


MM_LABELS = []


def _build_nc(causal: bool):
    nc = bacc.Bacc()
    MM_LABELS.clear()
    _orig_mm = nc.tensor.matmul
    _cur = {"l": "?"}

    def _mm(*a, **kw):
        MM_LABELS.append(_cur["l"])
        return _orig_mm(*a, **kw)
    nc.tensor.matmul = _mm

    def _lab(s):
        _cur["l"] = s
    nc._set_label = _lab

    hT3 = nc.declare_dram_parameter("hT3", [128, 8, S], BF16, isOutput=False)
    cs = nc.declare_dram_parameter("cs", [64, S], BF16, isOutput=False)
    sn = nc.declare_dram_parameter("sn", [64, S], BF16, isOutput=False)
    wq3 = nc.declare_dram_parameter("wq3", [128, 8, 256], BF16, isOutput=False)
    wkv3 = nc.declare_dram_parameter("wkv3", [128, 4, 256], BF16, isOutput=False)
    wo3 = nc.declare_dram_parameter("wo3", [128, 2, D], BF16, isOutput=False)
    psigT = nc.declare_dram_parameter("psigT", [128, 128], BF16, isOutput=False)
    ident = nc.declare_dram_parameter("ident", [128, 128], BF16, isOutput=False)
    m01 = nc.declare_dram_parameter("m01", [128, 128], BF16, isOutput=False)
    onesc = nc.declare_dram_parameter("onesc", [128, 64], BF16, isOutput=False)
    outp = nc.declare_dram_parameter("out", [S, D], BF16, isOutput=True)

    with tile.TileContext(nc) as tc, nc.allow_low_precision(
            reason="2e-2 rel tolerance; bf16 throughout"):
        with tc.tile_pool(name="hold", bufs=1) as hp:
            wkv_sb = hp.tile([128, 4, 256], BF16, name="wkv_sb", tag="wkv_sb")
            psig_sb = hp.tile([128, 128], BF16, name="psig_sb", tag="psig_sb")
            cosf = hp.tile([128, S], BF16, name="cosf", tag="cosf")
            sinf = hp.tile([128, S], BF16, name="sinf", tag="sinf")
            wq_sb = hp.tile([128, 8, 256], BF16, name="wq_sb", tag="wq_sb")
            id_sb = hp.tile([128, 128], BF16, name="id_sb", tag="id_sb")
            m01_sb = hp.tile([128, 128], BF16, name="m01_sb", tag="m01_sb")
            ones_sb = hp.tile([128, 64], BF16, name="ones_sb", tag="ones_sb")
            wo_sb = hp.tile([128, 2, D], BF16, name="wo_sb", tag="wo_sb")
            ht_sb = hp.tile([128, 8, S], BF16, name="ht_sb", tag="ht_sb")
            qTs = [hp.tile([128, S], BF16, name=f"qT{p}", tag=f"qT{p}")
                   for p in range(2)]
            kT = hp.tile([128, S], BF16, name="kTt", tag="kTt")
            vsm = hp.tile([128, NT, 65], BF16, name="vsm", tag="vsm")
            ctxTs = [[hp.tile([128, 512], BF16, name=f"ctxT{c}_{q}",
                              tag=f"ctxT{c}_{q}") for q in range(NQB)]
                     for c in range(2)]

            # ---- prologue DMAs (SP queue), ordered by first use ----
            nc.sync.dma_start(out=wkv_sb, in_=wkv3[:, :, :])
            nc.sync.dma_start(out=ht_sb[:, 0:4, 0:512], in_=hT3[:, 0:4, 0:512])
            nc.sync.dma_start(out=ht_sb[:, 4:8, 0:512], in_=hT3[:, 4:8, 0:512])
            nc.sync.dma_start(out=psig_sb, in_=psigT[:, :])
            nc.sync.dma_start(out=cosf[0:64, :], in_=cs[:, :])
            nc.sync.dma_start(out=cosf[64:128, :], in_=cs[:, :])
            nc.sync.dma_start(out=sinf[0:64, :], in_=sn[:, :])
            nc.sync.dma_start(out=sinf[64:128, :], in_=sn[:, :])
            nc.sync.dma_start(out=wq_sb, in_=wq3[:, :, :])
            nc.sync.dma_start(out=ht_sb[:, :, 512:1024], in_=hT3[:, :, 512:1024])
            nc.sync.dma_start(out=id_sb, in_=ident[:, :])
            nc.sync.dma_start(out=m01_sb, in_=m01[:, :])
            nc.sync.dma_start(out=ones_sb, in_=onesc[:, :])
            nc.sync.dma_start(out=wo_sb, in_=wo3[:, :, :])
            nc.sync.dma_start(out=ht_sb[:, :, 1024:1536], in_=hT3[:, :, 1024:1536])
            nc.sync.dma_start(out=ht_sb[:, :, 1536:2048], in_=hT3[:, :, 1536:2048])
            # ones column of vsm for the softmax denominator row
            nc.gpsimd.memset(vsm[:, :, 64:65], 1.0)
            # warm the ACT exp table while ACT is idle (the implicit load
            # would otherwise land on the first attention exp)
            actw = hp.tile([1, 16], F32, name="actw", tag="actw")
            nc.gpsimd.memset(actw[:, :], 1.0)
            nc.scalar.activation(actw, actw, AF.Exp, scale=1.0)

            with tc.tile_pool(name="psS", bufs=1, space="PSUM") as psS, \
                 tc.tile_pool(name="psD", bufs=1, space="PSUM") as psD, \
                 tc.tile_pool(name="psC", bufs=1, space="PSUM") as psC, \
                 tc.tile_pool(name="etp", bufs=1) as etp, \
                 tc.tile_pool(name="sbA", bufs=1) as sbA, \
                 tc.tile_pool(name="sbC", bufs=1) as sbC:

                # ---------------- phase A units (per strip) ----------------
                def u_kv(sc):
                    csl = slice(512 * sc, 512 * sc + 512)

                    st = {}

                    def f1():
                        ps = psD.tile([128, 512], F32, name="ps_kv",
                                      tag="ps_d", bufs=2)
                        for dc in range(4):
                            nc.tensor.matmul(
                                ps, wkv_sb[:, dc // 2,
                                           128 * (dc % 2):128 * (dc % 2) + 128],
                                ht_sb[:, dc, csl],
                                start=(dc == 0), stop=False)
                        st["ps"] = ps

                    def f2():
                        ps = st["ps"]
                        for dc in range(4, 8):
                            nc.tensor.matmul(
                                ps, wkv_sb[:, dc // 2,
                                           128 * (dc % 2):128 * (dc % 2) + 128],
                                ht_sb[:, dc, csl],
                                start=False, stop=(dc == 7))
                        kvraw = sbA.tile([128, 512], BF16, name="kvraw",
                                         tag="kvraw", bufs=2)
                        nc.vector.tensor_copy(kvraw, ps)
                        return kvraw
                    return f1, f2

                def u_kv_rope(sc, get):
                    csl = slice(512 * sc, 512 * sc + 512)

                    def f():
                        kvraw = get()
                        ps_kr = psD.tile([128, 512], F32, name="ps_kr",
                                         tag="ps_d", bufs=2)[0:64, :]
                        nc.tensor.matmul(ps_kr, psig_sb[0:64, 0:64],
                                         kvraw[0:64, :], start=True, stop=True)
                        kdst = kT[0:64, csl]
                        nc.gpsimd.tensor_mul(kdst, kvraw[0:64, :],
                                             cosf[0:64, csl])
                        ktmp = sbA.tile([64, 512], BF16, name="ktmp",
                                        tag="ktmp", bufs=2)
                        nc.vector.tensor_mul(ktmp, ps_kr, sinf[0:64, csl])
                        nc.gpsimd.tensor_add(kdst, kdst, ktmp)
                        # duplicate roped K to partitions 64:128 (odd heads)
                        nc.sync.dma_start(out=kT[64:128, csl], in_=kdst)
                    return f

                def u_v_t(sc, get):
                    def f():
                        kvraw = get()
                        vst = sbA.tile([128, 4, 64], BF16, name="vst",
                                       tag="vst", bufs=2)
                        nc.sync.dma_start_transpose(
                            out=vst[:, :, :], in_=kvraw[64:128, :])
                        nc.vector.tensor_copy(vsm[:, 4 * sc:4 * sc + 4, 0:64],
                                              vst[:, :, :])
                    return f

                def u_q(sc, pp):
                    csl = slice(512 * sc, 512 * sc + 512)

                    st = {}

                    def f1():
                        ps = psD.tile([128, 512], F32, name="ps_q",
                                      tag="ps_d", bufs=2)
                        for dc in range(4):
                            nc.tensor.matmul(
                                ps, wq_sb[:, dc, 128 * pp:128 * pp + 128],
                                ht_sb[:, dc, csl],
                                start=(dc == 0), stop=False)
                        st["ps"] = ps

                    def f2():
                        ps = st["ps"]
                        for dc in range(4, 8):
                            nc.tensor.matmul(
                                ps, wq_sb[:, dc, 128 * pp:128 * pp + 128],
                                ht_sb[:, dc, csl],
                                start=False, stop=(dc == 7))
                        qraw = sbA.tile([128, 512], BF16, name="qraw",
                                        tag="qraw", bufs=2)
                        nc.vector.tensor_copy(qraw, ps)
                        return qraw
                    return f1, f2

                def u_q_rope(sc, pp, get):
                    csl = slice(512 * sc, 512 * sc + 512)

                    def f():
                        qraw = get()
                        ps_rot = psD.tile([128, 512], F32, name="ps_rot",
                                          tag="ps_d", bufs=2)
                        nc.tensor.matmul(ps_rot, psig_sb, qraw,
                                         start=True, stop=True)
                        dst = qTs[pp][:, csl]
                        nc.gpsimd.tensor_mul(dst, qraw, cosf[:, csl])
                        rtmp = sbA.tile([128, 512], BF16, name="rtmp",
                                        tag="rtmp", bufs=2)
                        nc.vector.tensor_mul(rtmp, ps_rot, sinf[:, csl])
                        nc.gpsimd.tensor_add(dst, dst, rtmp)
                    return f

                def strip_units(sc):
                    box = {}

                    def mk(key, fn):
                        def g():
                            box[key] = fn()
                        return g

                    def rd(key):
                        return lambda: box[key]

                    kv1, kv2 = u_kv(sc)
                    q01, q02 = u_q(sc, 0)
                    q11, q12 = u_q(sc, 1)
                    return [
                        kv1,
                        mk("kv", kv2),
                        q01,
                        mk("q0", q02),
                        u_kv_rope(sc, rd("kv")),
                        u_q_rope(sc, 0, rd("q0")),
                        u_v_t(sc, rd("kv")),
                        q11,
                        mk("q1", q12),
                        u_q_rope(sc, 1, rd("q1")),
                    ]

                # ---------------- attention units ----------------
                # per (qb, sp): heads hA=2sp, hB=2sp+1 interleaved
                def bc_units(qb):
                    qsl = slice(512 * qb, 512 * qb + 512)
                    units = []
                    sps = (1, 0) if qb == NQB - 1 else (0, 1)
                    for sp in sps:
                        hs = [2 * sp, 2 * sp + 1]
                        ctxps = {}
                        nki = (4 * qb + 4) if causal else NT
                        nfull = (4 * qb) if causal else NT

                        def mk_ctx_alloc(h):
                            def g():
                                ctxps[h] = psC.tile([128, 512], F32,
                                                    name="ps_ctx",
                                                    tag="ps_ctx", bufs=2)
                            return g

                        # scores+exp+ctx closures
                        def mk_pair_s(h, kp, bx):
                            off = 64 * (h % 2)
                            pp = h // 2

                            def g():
                                ps = psS.tile([128, 1024], F32, name="ps_s",
                                              tag="ps_s", bufs=2)
                                for jj in range(2):
                                    ki = kp + jj
                                    nc.tensor.matmul(
                                        ps[:, 512 * jj:512 * jj + 512],
                                        kT[off:off + 64,
                                           128 * ki:128 * ki + 128],
                                        qTs[pp][off:off + 64, qsl],
                                        start=True, stop=True)
                                bx["ps"] = ps
                            return g

                        def mk_pair_e(h, kp, bx, _nki=nki):
                            def g():
                                et = etp.tile([128, 1024], BF16, name="et",
                                              tag="et", bufs=6)
                                nc.scalar.activation(et, bx["ps"], AF.Exp,
                                                     scale=SCALE)
                                for jj in range(2):
                                    ki = kp + jj
                                    nc.tensor.matmul(
                                        ctxps[h][0:65, :],
                                        vsm[:, ki, 0:65],
                                        et[:, 512 * jj:512 * jj + 512],
                                        start=(ki == 0),
                                        stop=(ki == _nki - 1))
                            return g

                        # diag group: js = (0,1) or (2,3); slot offsets
                        def mk_diag_s(h, js, bx):
                            off = 64 * (h % 2)
                            pp = h // 2

                            def g():
                                spans = [512 - 128 * j for j in js]
                                tot = sum(spans)
                                ps = psS.tile([128, 1024], F32, name="ps_g",
                                              tag="ps_s", bufs=2)
                                o = 0
                                for j, span in zip(js, spans):
                                    ki = 4 * qb + j
                                    # diagonal 128-col block: scores then the
                                    # folded causal mask (+= I^T @ -1e9 tri)
                                    nc.tensor.matmul(
                                        ps[:, o:o + 128],
                                        kT[off:off + 64,
                                           128 * ki:128 * ki + 128],
                                        qTs[pp][off:off + 64,
                                                512 * qb + 128 * j:
                                                512 * qb + 128 * j + 128],
                                        start=True, stop=False)
                                    nc.tensor.matmul(
                                        ps[:, o:o + 128], id_sb, m01_sb,
                                        start=False, stop=True)
                                    if span > 128:
                                        # fully-causal remainder of the span
                                        nc.tensor.matmul(
                                            ps[:, o + 128:o + span],
                                            kT[off:off + 64,
                                               128 * ki:128 * ki + 128],
                                            qTs[pp][off:off + 64,
                                                    512 * qb + 128 * (j + 1):
                                                    512 * qb + 512],
                                            start=True, stop=True)
                                    o += span
                                bx["ps"] = ps
                                bx["tot"] = tot
                            return g

                        def mk_diag_e(h, js, bx, _nki=nki):
                            def g():
                                spans = [512 - 128 * j for j in js]
                                et = etp.tile([128, 1024], BF16, name="etd",
                                              tag="et", bufs=6)
                                nc.scalar.activation(et[:, 0:bx["tot"]],
                                                     bx["ps"][:, 0:bx["tot"]],
                                                     AF.Exp, scale=SCALE)
                                o = 0
                                for j, span in zip(js, spans):
                                    ki = 4 * qb + j
                                    nc.tensor.matmul(
                                        ctxps[h][0:65, 128 * j:512],
                                        vsm[:, ki, 0:65],
                                        et[:, o:o + span],
                                        start=(ki == 0),
                                        stop=(ki == _nki - 1))
                                    o += span
                            return g

                        def mk_norm(h):
                            c = h % 2
                            up = h // 2   # 0: partitions 0:64, 1: 64:128

                            def g():
                                pc = ctxps[h]
                                ctxu = sbC.tile([65, 512], BF16, name="ctxu",
                                                tag="ctxu", bufs=3)
                                nc.vector.tensor_copy(ctxu, pc[0:65, :])
                                nc.vector.reciprocal(ctxu[64:65, :],
                                                     ctxu[64:65, :])
                                ps_b = psD.tile([128, 512], F32, name="ps_b",
                                                tag="ps_d", bufs=2)[0:64, :]
                                nc.tensor.matmul(
                                    ps_b, ones_sb[64:65, :],
                                    ctxu[64:65, :],
                                    start=True, stop=True)
                                if up == 0:
                                    nc.vector.tensor_mul(
                                        ctxTs[c][qb][0:64, :],
                                        ctxu[0:64, :], ps_b)
                                else:
                                    ctmp = sbC.tile([64, 512], BF16,
                                                    name="ctmp", tag="ctmp",
                                                    bufs=2)
                                    nc.vector.tensor_mul(ctmp, ctxu[0:64, :],
                                                         ps_b)
                                    nc.sync.dma_start(
                                        out=ctxTs[c][qb][64:128, :], in_=ctmp)
                            return g

                        # build interleaved 2-head stream for this sp
                        hA, hB = hs
                        units.append(mk_ctx_alloc(hA))
                        units.append(mk_ctx_alloc(hB))
                        boxes = {h: [] for h in hs}
                        sA, eA, sB, eB = [], [], [], []
                        for kp in range(0, nfull, 2):
                            bxA, bxB = {}, {}
                            sA.append(mk_pair_s(hA, kp, bxA))
                            eA.append(mk_pair_e(hA, kp, bxA))
                            sB.append(mk_pair_s(hB, kp, bxB))
                            eB.append(mk_pair_e(hB, kp, bxB))
                        if causal:
                            for js in ((0, 1), (2, 3)):
                                bxA, bxB = {}, {}
                                sA.append(mk_diag_s(hA, js, bxA))
                                eA.append(mk_diag_e(hA, js, bxA))
                                sB.append(mk_diag_s(hB, js, bxB))
                                eB.append(mk_diag_e(hB, js, bxB))
                        # pipeline: sA0 sB0 | eA0 sA1 eB0 sB1 | eA1 sA2 ...
                        n = len(sA)
                        if n:
                            units.append(sA[0])
                            units.append(sB[0])
                            for i in range(n):
                                if i + 1 < n:
                                    units.append(eA[i])
                                    units.append(sA[i + 1])
                                    units.append(eB[i])
                                    units.append(sB[i + 1])
                                else:
                                    units.append(eA[i])
                                    units.append(eB[i])
                        units.append(mk_norm(hA))
                        units.append(mk_norm(hB))
                    return units

                # ---------------- phase D units ----------------
                def d_units(qb, use_act=False):
                    units = []
                    for qt in range(4):
                        def mk(qt=qt):
                            rows = slice(512 * qb + 128 * qt,
                                         512 * qb + 128 * qt + 128)
                            col = 128 * qt

                            def g():
                                ost = sbC.tile([128, 1024], BF16, name="ost",
                                               tag="ost", bufs=3)
                                split = use_act
                                for nb in range(2):
                                    ps_o = psD.tile([128, 512], F32,
                                                    name="ps_o", tag="ps_d",
                                                    bufs=2)
                                    for c in range(2):
                                        nc.tensor.matmul(
                                            ps_o,
                                            ctxTs[c][qb][:, col:col + 128],
                                            wo_sb[:, c,
                                                  512 * nb:512 * nb + 512],
                                            start=(c == 0), stop=(c == 1))
                                    dst = ost[:, 512 * nb:512 * nb + 512]
                                    if split and nb == 0:
                                        nc.scalar.copy(dst, ps_o)
                                    else:
                                        nc.vector.tensor_copy(dst, ps_o)
                                    if split:
                                        nc.sync.dma_start(
                                            out=outp[rows,
                                                     512 * nb:512 * nb + 512],
                                            in_=dst)
                                if not split:
                                    nc.sync.dma_start(out=outp[rows, :],
                                                      in_=ost)
                            return g
                        units.append(mk())
                    return units

                # ---------------- global emission ----------------
                def lab_units(units, pfx):
                    out = []
                    for i, u in enumerate(units):
                        def w(u=u, l=f"{pfx}.{i}"):
                            nc._set_label(l)
                            u()
                        out.append(w)
                    return out

                for u in lab_units(strip_units(0), "strip0"):
                    u()
                for qb in range(NQB):
                    bcu = lab_units(bc_units(qb), f"bc{qb}")
                    fill = []
                    if qb == 0:
                        fill += lab_units(strip_units(1), "strip1")
                    if qb == 1:
                        fill += lab_units(strip_units(2), "strip2")
                    if qb == 2:
                        fill += lab_units(strip_units(3), "strip3")
                        fill += lab_units(d_units(0), "d0")
                    if qb == 3:
                        fill += lab_units(d_units(1), "d1")
                        fill += lab_units(d_units(2), "d2")
                    for u in _interleave(bcu, fill):
                        u()
                for u in lab_units(d_units(3, use_act=True), "d3"):
                    u()

    nc.compile()
    return nc


_NC_CACHE = {}


def _get_nc(causal: bool):
    if causal not in _NC_CACHE:
        _NC_CACHE[causal] = _build_nc(causal)
    return _NC_CACHE[causal]


def _host_consts():
    p = np.zeros((128, 128), np.float32)
    idx = np.arange(0, 128, 2)
    p[idx, idx + 1] = -1.0
    p[idx + 1, idx] = 1.0
    psigT = np.ascontiguousarray(p.T)
    ident = np.eye(128, dtype=np.float32)
    m01n = np.where(np.arange(128)[None, :] >= np.arange(128)[:, None],
                    0.0, NEG).astype(np.float32)
    return psigT, ident, m01n


def _numpy_reference(hidden_states, cos, sin, attention_mask, Wq, Wk, Wv, Wo):
    """Generic-mask fallback, pure numpy port of the reference."""
    GROUPS = H // KVH

    def rope(x, c, s):
        c = c[:, None, :, :]
        s = s[:, None, :, :]
        x1, x2 = x[..., ::2], x[..., 1::2]
        xr = np.stack([x1 * c - x2 * s, x1 * s + x2 * c], axis=-1)
        return xr.reshape(x.shape)

    b, sq, d = hidden_states.shape
    q = (hidden_states @ Wq).reshape(b, sq, H, HD).transpose(0, 2, 1, 3)
    k = (hidden_states @ Wk).reshape(b, sq, KVH, HD).transpose(0, 2, 1, 3)
    v = (hidden_states @ Wv).reshape(b, sq, KVH, HD).transpose(0, 2, 1, 3)
    q = rope(q, cos, sin)
    k = rope(k, cos, sin)
    k = np.repeat(k, GROUPS, axis=1)
    v = np.repeat(v, GROUPS, axis=1)
    out = np.zeros((b, sq, d), np.float32)
    for bi in range(b):
        for hi in range(H):
            sc = (q[bi, hi] @ k[bi, hi].T) * SCALE + attention_mask[0, 0]
            sc = sc - sc.max(axis=-1, keepdims=True)
            e = np.exp(sc)
            pr = e / e.sum(axis=-1, keepdims=True)
            ctx = pr @ v[bi, hi]
            out[bi] += ctx @ Wo[hi * HD:(hi + 1) * HD]
    return out


def kernel(**inputs) -> np.ndarray:
    hs = np.asarray(inputs["hidden_states"], np.float32)
    cos = np.asarray(inputs["cos"], np.float32)
    sin = np.asarray(inputs["sin"], np.float32)
    mask = np.asarray(inputs["attention_mask"], np.float32)
    Wq = np.asarray(inputs["Wq"], np.float32)
    Wk = np.asarray(inputs["Wk"], np.float32)
    Wv = np.asarray(inputs["Wv"], np.float32)
    Wo = np.asarray(inputs["Wo"], np.float32)

    m = mask.reshape(S, S)
    tril = np.tril(np.ones((S, S), dtype=bool))
    causal_ref = np.where(tril, np.float32(0.0), np.float32(NEG))
    if np.array_equal(m, causal_ref):
        causal = True
    elif not m.any():
        causal = False
    else:
        return _numpy_reference(hs, cos, sin, mask, Wq, Wk, Wv, Wo)

    nc = _get_nc(causal)
    psigT, ident, m01 = _host_consts()
    chan_half = (np.arange(64) // 2)
    bf = ml_dtypes.bfloat16

    in_maps = []
    for core in range(8):
        b, t = core // TP, core % TP
        hT = np.ascontiguousarray(hs[b].T).astype(bf)
        hT3 = np.ascontiguousarray(
            hT.reshape(8, 128, S).transpose(1, 0, 2))
        cs_v = np.ascontiguousarray(cos[b].T[chan_half, :]).astype(bf)
        sn_v = np.ascontiguousarray(sin[b].T[chan_half, :]).astype(bf)
        wq_s = Wq[:, t * 256:(t + 1) * 256].astype(bf)
        wq3 = np.ascontiguousarray(wq_s.reshape(8, 128, 256).transpose(1, 0, 2))
        wkv_s = np.concatenate([Wk[:, t * 64:(t + 1) * 64],
                                Wv[:, t * 64:(t + 1) * 64]], axis=1).astype(bf)
        wkv3 = np.ascontiguousarray(
            wkv_s.reshape(8, 128, 128).transpose(1, 0, 2)
            .reshape(128, 4, 256))
        wo_s = Wo[t * 256:(t + 1) * 256]
        # ctxT channel order per chunk: c0 = [h0|h2], c1 = [h1|h3]
        wo_p = np.concatenate([wo_s[0:64], wo_s[128:192],
                               wo_s[64:128], wo_s[192:256]], axis=0).astype(bf)
        wo3 = np.ascontiguousarray(wo_p.reshape(2, 128, D).transpose(1, 0, 2))
        in_maps.append({
            "hT3": hT3, "cs": cs_v, "sn": sn_v,
            "wq3": wq3, "wkv3": wkv3, "wo3": wo3,
            "psigT": psigT.astype(bf), "ident": ident.astype(bf),
            "m01": m01.astype(bf),
            "onesc": np.ones((128, 64), bf),
        })

    res = run_bass_kernel_spmd(nc, in_maps, core_ids=list(range(8)))
    out = np.zeros((B, S, D), np.float32)
    for core in range(8):
        out[core // TP] += np.asarray(res.results[core]["out"],
                                      dtype=np.float32)
    return out


# revision 3
# speedup vs baseline: 1.0116x; 1.0116x over previous
"""Self-contained Trainium2 Bass kernel for GQA MultiHeadAttention with RoPE.

Problem: B=2, S=2048, D=1024, H=16 Q heads, KVH=4 KV heads, head_dim=64,
causal additive mask, f32 in/out.

Sharding: TP=4 over heads (4 Q heads + 1 KV head per shard) x DP=2 over
batch = 8 NeuronCores. Wo sharded on input dim; host sums the 4 partial
outputs per batch element.

Design notes (v2):
- All-bf16 datapath (weights, hidden, q/k/v, exp probs, ctx, out partials);
  psum accumulation stays f32. Error budget 2e-2 rel leaves ~7x headroom.
- Phase A (projections+rope) is emitted strip-wise (512 seq cols) and
  interleaved with attention blocks so PE never idles on exp latency.
- Softmax normalization is fused on-psum: DVE reciprocal of the rowsum row
  (row 64 of the ctx psum, produced by an ones-column in vsm), PE ones-matmul
  broadcast of the recip row, DVE multiply straight out of psum into ctxT.
  No DRAM scratch, no reshape bounces.
- Diagonal (causal-edge) tiles are batched into 2 grouped psum slots per
  (head, qb) so exps are [128, 896]+[128, 384] instead of 4 small ones.
- DMAs are coalesced (one per hT strip, one out-store per 128-row tile) and
  issued on the engine that produced their source.
"""

import os
import sys

for _p in ("/opt/trn_rl_repo", "/root/.axon_site/_ro/trn_rl_repo"):
    if os.path.isdir(_p) and _p not in sys.path:
        sys.path.insert(0, _p)

import numpy as np
import ml_dtypes

import concourse.bacc as bacc
import concourse.bass as bass
import concourse.tile as tile
from concourse import mybir
from concourse.bass_utils import run_bass_kernel_spmd

F32 = mybir.dt.float32
F32R = mybir.dt.float32r
BF16 = mybir.dt.bfloat16
AF = mybir.ActivationFunctionType

H, KVH, HD = 16, 4, 64
B, S, D = 2, 2048, 1024
TP = 4
SCALE = HD ** -0.5
NEG = -1e9
NT = S // 128               # 16 kv tiles
NQB = S // 512              # 4 q blocks


def _interleave(a, b):
    """Merge two unit lists proportionally (a is the primary stream)."""
    if not b:
        return list(a)
    if not a:
        return list(b)
    out = []
    na, nb = len(a), len(b)
    ia = ib = 0
    while ia < na or ib < nb:
        # emit whichever stream is behind proportionally
        if ib >= nb or (ia < na and ia * nb <= ib * na):
            out.append(a[ia]); ia += 1
        else:
            out.append(b[ib]); ib += 1
    return out


MM_LABELS = []


def _build_nc(causal: bool):
    nc = bacc.Bacc()
    MM_LABELS.clear()
    _orig_mm = nc.tensor.matmul
    _cur = {"l": "?"}

    def _mm(*a, **kw):
        MM_LABELS.append(_cur["l"])
        return _orig_mm(*a, **kw)
    nc.tensor.matmul = _mm

    def _lab(s):
        _cur["l"] = s
    nc._set_label = _lab

    hT3 = nc.declare_dram_parameter("hT3", [128, 8, S], BF16, isOutput=False)
    cs = nc.declare_dram_parameter("cs", [64, S], BF16, isOutput=False)
    sn = nc.declare_dram_parameter("sn", [64, S], BF16, isOutput=False)
    wq3 = nc.declare_dram_parameter("wq3", [128, 8, 256], BF16, isOutput=False)
    wkv3 = nc.declare_dram_parameter("wkv3", [128, 4, 256], BF16, isOutput=False)
    wo3 = nc.declare_dram_parameter("wo3", [128, 2, D], BF16, isOutput=False)
    psigT = nc.declare_dram_parameter("psigT", [128, 128], BF16, isOutput=False)
    ident = nc.declare_dram_parameter("ident", [128, 128], BF16, isOutput=False)
    m01 = nc.declare_dram_parameter("m01", [128, 128], BF16, isOutput=False)
    onesc = nc.declare_dram_parameter("onesc", [128, 64], BF16, isOutput=False)
    outp = nc.declare_dram_parameter("out", [S, D], BF16, isOutput=True)

    with tile.TileContext(nc) as tc, nc.allow_low_precision(
            reason="2e-2 rel tolerance; bf16 throughout"):
        with tc.tile_pool(name="hold", bufs=1) as hp:
            wkv_sb = hp.tile([128, 4, 256], BF16, name="wkv_sb", tag="wkv_sb")
            psig_sb = hp.tile([128, 128], BF16, name="psig_sb", tag="psig_sb")
            cosf = hp.tile([128, S], BF16, name="cosf", tag="cosf")
            sinf = hp.tile([128, S], BF16, name="sinf", tag="sinf")
            wq_sb = hp.tile([128, 8, 256], BF16, name="wq_sb", tag="wq_sb")
            id_sb = hp.tile([128, 128], BF16, name="id_sb", tag="id_sb")
            m01_sb = hp.tile([128, 128], BF16, name="m01_sb", tag="m01_sb")
            ones_sb = hp.tile([128, 64], BF16, name="ones_sb", tag="ones_sb")
            wo_sb = hp.tile([128, 2, D], BF16, name="wo_sb", tag="wo_sb")
            ht_sb = hp.tile([128, 8, S], BF16, name="ht_sb", tag="ht_sb")
            qTs = [hp.tile([128, S], BF16, name=f"qT{p}", tag=f"qT{p}")
                   for p in range(2)]
            kT = hp.tile([128, S], BF16, name="kTt", tag="kTt")
            vsm = hp.tile([128, NT, 65], BF16, name="vsm", tag="vsm")
            ctxTs = [[hp.tile([128, 512], BF16, name=f"ctxT{c}_{q}",
                              tag=f"ctxT{c}_{q}") for q in range(NQB)]
                     for c in range(2)]

            # ---- prologue DMAs (SP queue), ordered by first use ----
            nc.sync.dma_start(out=wkv_sb, in_=wkv3[:, :, :])
            nc.sync.dma_start(out=ht_sb[:, 0:4, 0:512], in_=hT3[:, 0:4, 0:512])
            nc.sync.dma_start(out=ht_sb[:, 4:8, 0:512], in_=hT3[:, 4:8, 0:512])
            nc.sync.dma_start(out=psig_sb, in_=psigT[:, :])
            nc.sync.dma_start(out=cosf[0:64, :], in_=cs[:, :])
            nc.sync.dma_start(out=cosf[64:128, :], in_=cs[:, :])
            nc.sync.dma_start(out=sinf[0:64, :], in_=sn[:, :])
            nc.sync.dma_start(out=sinf[64:128, :], in_=sn[:, :])
            nc.sync.dma_start(out=wq_sb, in_=wq3[:, :, :])
            nc.sync.dma_start(out=ht_sb[:, :, 512:1024], in_=hT3[:, :, 512:1024])
            nc.sync.dma_start(out=id_sb, in_=ident[:, :])
            nc.sync.dma_start(out=m01_sb, in_=m01[:, :])
            nc.sync.dma_start(out=ones_sb, in_=onesc[:, :])
            nc.sync.dma_start(out=wo_sb, in_=wo3[:, :, :])
            nc.sync.dma_start(out=ht_sb[:, :, 1024:1536], in_=hT3[:, :, 1024:1536])
            nc.sync.dma_start(out=ht_sb[:, :, 1536:2048], in_=hT3[:, :, 1536:2048])
            # ones column of vsm for the softmax denominator row
            nc.gpsimd.memset(vsm[:, :, 64:65], 1.0)
            # warm the ACT exp table while ACT is idle (the implicit load
            # would otherwise land on the first attention exp)
            actw = hp.tile([1, 16], F32, name="actw", tag="actw")
            nc.gpsimd.memset(actw[:, :], 1.0)
            nc.scalar.activation(actw, actw, AF.Exp, scale=1.0)

            with tc.tile_pool(name="psS", bufs=1, space="PSUM") as psS, \
                 tc.tile_pool(name="psD", bufs=1, space="PSUM") as psD, \
                 tc.tile_pool(name="psC", bufs=1, space="PSUM") as psC, \
                 tc.tile_pool(name="etp", bufs=1) as etp, \
                 tc.tile_pool(name="sbA", bufs=1) as sbA, \
                 tc.tile_pool(name="sbC", bufs=1) as sbC:

                # ---------------- phase A units (per strip) ----------------
                def u_kv(sc):
                    csl = slice(512 * sc, 512 * sc + 512)

                    st = {}

                    def f1():
                        ps = psD.tile([128, 512], F32, name="ps_kv",
                                      tag="ps_d", bufs=2)
                        for dc in range(4):
                            nc.tensor.matmul(
                                ps, wkv_sb[:, dc // 2,
                                           128 * (dc % 2):128 * (dc % 2) + 128],
                                ht_sb[:, dc, csl],
                                start=(dc == 0), stop=False)
                        st["ps"] = ps

                    def f2():
                        ps = st["ps"]
                        for dc in range(4, 8):
                            nc.tensor.matmul(
                                ps, wkv_sb[:, dc // 2,
                                           128 * (dc % 2):128 * (dc % 2) + 128],
                                ht_sb[:, dc, csl],
                                start=False, stop=(dc == 7))
                        kvraw = sbA.tile([128, 512], BF16, name="kvraw",
                                         tag="kvraw", bufs=3)
                        nc.vector.tensor_copy(kvraw, ps)
                        return kvraw
                    return f1, f2

                def u_kv_rope(sc, get):
                    csl = slice(512 * sc, 512 * sc + 512)

                    def f():
                        kvraw = get()
                        ps_kr = psD.tile([128, 512], F32, name="ps_kr",
                                         tag="ps_d", bufs=2)[0:64, :]
                        nc.tensor.matmul(ps_kr, psig_sb[0:64, 0:64],
                                         kvraw[0:64, :], start=True, stop=True)
                        kdst = kT[0:64, csl]
                        nc.gpsimd.tensor_mul(kdst, kvraw[0:64, :],
                                             cosf[0:64, csl])
                        ktmp = sbA.tile([64, 512], BF16, name="ktmp",
                                        tag="ktmp", bufs=2)
                        nc.vector.tensor_mul(ktmp, ps_kr, sinf[0:64, csl])
                        nc.gpsimd.tensor_add(kdst, kdst, ktmp)
                        # duplicate roped K to partitions 64:128 (odd heads)
                        nc.sync.dma_start(out=kT[64:128, csl], in_=kdst)
                    return f

                def u_v_t(sc, get):
                    def f():
                        kvraw = get()
                        vst = sbA.tile([128, 4, 64], BF16, name="vst",
                                       tag="vst", bufs=2)
                        nc.sync.dma_start_transpose(
                            out=vst[:, :, :], in_=kvraw[64:128, :])
                        nc.vector.tensor_copy(vsm[:, 4 * sc:4 * sc + 4, 0:64],
                                              vst[:, :, :])
                    return f

                def u_q(sc, pp):
                    csl = slice(512 * sc, 512 * sc + 512)

                    st = {}

                    def f1():
                        ps = psD.tile([128, 512], F32, name="ps_q",
                                      tag="ps_d", bufs=2)
                        for dc in range(4):
                            nc.tensor.matmul(
                                ps, wq_sb[:, dc, 128 * pp:128 * pp + 128],
                                ht_sb[:, dc, csl],
                                start=(dc == 0), stop=False)
                        st["ps"] = ps

                    def f2():
                        ps = st["ps"]
                        for dc in range(4, 8):
                            nc.tensor.matmul(
                                ps, wq_sb[:, dc, 128 * pp:128 * pp + 128],
                                ht_sb[:, dc, csl],
                                start=False, stop=(dc == 7))
                        qraw = sbA.tile([128, 512], BF16, name="qraw",
                                        tag="qraw", bufs=3)
                        nc.vector.tensor_copy(qraw, ps)
                        return qraw
                    return f1, f2

                def u_q_rope(sc, pp, get):
                    csl = slice(512 * sc, 512 * sc + 512)

                    def f():
                        qraw = get()
                        ps_rot = psD.tile([128, 512], F32, name="ps_rot",
                                          tag="ps_d", bufs=2)
                        nc.tensor.matmul(ps_rot, psig_sb, qraw,
                                         start=True, stop=True)
                        dst = qTs[pp][:, csl]
                        nc.gpsimd.tensor_mul(dst, qraw, cosf[:, csl])
                        rtmp = sbA.tile([128, 512], BF16, name="rtmp",
                                        tag="rtmp", bufs=3)
                        nc.vector.tensor_mul(rtmp, ps_rot, sinf[:, csl])
                        nc.gpsimd.tensor_add(dst, dst, rtmp)
                    return f

                def strip_units(sc):
                    box = {}

                    def mk(key, fn):
                        def g():
                            box[key] = fn()
                        return g

                    def rd(key):
                        return lambda: box[key]

                    kv1, kv2 = u_kv(sc)
                    q01, q02 = u_q(sc, 0)
                    q11, q12 = u_q(sc, 1)
                    return [
                        kv1,
                        mk("kv", kv2),
                        q01,
                        mk("q0", q02),
                        u_kv_rope(sc, rd("kv")),
                        u_q_rope(sc, 0, rd("q0")),
                        u_v_t(sc, rd("kv")),
                        q11,
                        mk("q1", q12),
                        u_q_rope(sc, 1, rd("q1")),
                    ]

                # ---------------- attention units ----------------
                # per (qb, sp): heads hA=2sp, hB=2sp+1 interleaved
                def bc_units(qb):
                    qsl = slice(512 * qb, 512 * qb + 512)
                    units = []
                    sps = (1, 0) if qb == NQB - 1 else (0, 1)
                    for sp in sps:
                        hs = [2 * sp, 2 * sp + 1]
                        ctxps = {}
                        nki = (4 * qb + 4) if causal else NT
                        nfull = (4 * qb) if causal else NT

                        def mk_ctx_alloc(h):
                            def g():
                                ctxps[h] = psC.tile([128, 512], F32,
                                                    name="ps_ctx",
                                                    tag="ps_ctx", bufs=2)
                            return g

                        # scores+exp+ctx closures
                        def mk_pair_s(h, kp, bx):
                            off = 64 * (h % 2)
                            pp = h // 2

                            def g():
                                ps = psS.tile([128, 1024], F32, name="ps_s",
                                              tag="ps_s", bufs=2)
                                for jj in range(2):
                                    ki = kp + jj
                                    nc.tensor.matmul(
                                        ps[:, 512 * jj:512 * jj + 512],
                                        kT[off:off + 64,
                                           128 * ki:128 * ki + 128],
                                        qTs[pp][off:off + 64, qsl],
                                        start=True, stop=True)
                                bx["ps"] = ps
                            return g

                        def mk_pair_e(h, kp, bx, _nki=nki):
                            def g():
                                et = etp.tile([128, 1024], BF16, name="et",
                                              tag="et", bufs=6)
                                nc.scalar.activation(et, bx["ps"], AF.Exp,
                                                     scale=SCALE)
                                for jj in range(2):
                                    ki = kp + jj
                                    nc.tensor.matmul(
                                        ctxps[h][0:65, :],
                                        vsm[:, ki, 0:65],
                                        et[:, 512 * jj:512 * jj + 512],
                                        start=(ki == 0),
                                        stop=(ki == _nki - 1))
                            return g

                        # diag group: js = (0,1) or (2,3); slot offsets
                        def mk_diag_s(h, js, bx):
                            off = 64 * (h % 2)
                            pp = h // 2

                            def g():
                                spans = [512 - 128 * j for j in js]
                                tot = sum(spans)
                                ps = psS.tile([128, 1024], F32, name="ps_g",
                                              tag="ps_s", bufs=2)
                                o = 0
                                for j, span in zip(js, spans):
                                    ki = 4 * qb + j
                                    # diagonal 128-col block: scores then the
                                    # folded causal mask (+= I^T @ -1e9 tri)
                                    nc.tensor.matmul(
                                        ps[:, o:o + 128],
                                        kT[off:off + 64,
                                           128 * ki:128 * ki + 128],
                                        qTs[pp][off:off + 64,
                                                512 * qb + 128 * j:
                                                512 * qb + 128 * j + 128],
                                        start=True, stop=False)
                                    nc.tensor.matmul(
                                        ps[:, o:o + 128], id_sb, m01_sb,
                                        start=False, stop=True)
                                    if span > 128:
                                        # fully-causal remainder of the span
                                        nc.tensor.matmul(
                                            ps[:, o + 128:o + span],
                                            kT[off:off + 64,
                                               128 * ki:128 * ki + 128],
                                            qTs[pp][off:off + 64,
                                                    512 * qb + 128 * (j + 1):
                                                    512 * qb + 512],
                                            start=True, stop=True)
                                    o += span
                                bx["ps"] = ps
                                bx["tot"] = tot
                            return g

                        def mk_diag_e(h, js, bx, _nki=nki):
                            def g():
                                spans = [512 - 128 * j for j in js]
                                et = etp.tile([128, 1024], BF16, name="etd",
                                              tag="et", bufs=6)
                                nc.scalar.activation(et[:, 0:bx["tot"]],
                                                     bx["ps"][:, 0:bx["tot"]],
                                                     AF.Exp, scale=SCALE)
                                o = 0
                                for j, span in zip(js, spans):
                                    ki = 4 * qb + j
                                    nc.tensor.matmul(
                                        ctxps[h][0:65, 128 * j:512],
                                        vsm[:, ki, 0:65],
                                        et[:, o:o + span],
                                        start=(ki == 0),
                                        stop=(ki == _nki - 1))
                                    o += span
                            return g

                        def mk_norm(h):
                            c = h % 2
                            up = h // 2   # 0: partitions 0:64, 1: 64:128

                            def g():
                                pc = ctxps[h]
                                ctxu = sbC.tile([65, 512], BF16, name="ctxu",
                                                tag="ctxu", bufs=4)
                                nc.vector.tensor_copy(ctxu, pc[0:65, :])
                                nc.vector.reciprocal(ctxu[64:65, :],
                                                     ctxu[64:65, :])
                                ps_b = psD.tile([128, 512], F32, name="ps_b",
                                                tag="ps_d", bufs=2)[0:64, :]
                                nc.tensor.matmul(
                                    ps_b, ones_sb[64:65, :],
                                    ctxu[64:65, :],
                                    start=True, stop=True)
                                if up == 0:
                                    nc.vector.tensor_mul(
                                        ctxTs[c][qb][0:64, :],
                                        ctxu[0:64, :], ps_b)
                                else:
                                    ctmp = sbC.tile([64, 512], BF16,
                                                    name="ctmp", tag="ctmp",
                                                    bufs=2)
                                    nc.vector.tensor_mul(ctmp, ctxu[0:64, :],
                                                         ps_b)
                                    nc.sync.dma_start(
                                        out=ctxTs[c][qb][64:128, :], in_=ctmp)
                            return g

                        # build interleaved 2-head stream for this sp
                        hA, hB = hs
                        units.append(mk_ctx_alloc(hA))
                        units.append(mk_ctx_alloc(hB))
                        boxes = {h: [] for h in hs}
                        sA, eA, sB, eB = [], [], [], []
                        for kp in range(0, nfull, 2):
                            bxA, bxB = {}, {}
                            sA.append(mk_pair_s(hA, kp, bxA))
                            eA.append(mk_pair_e(hA, kp, bxA))
                            sB.append(mk_pair_s(hB, kp, bxB))
                            eB.append(mk_pair_e(hB, kp, bxB))
                        if causal:
                            for js in ((0, 1), (2, 3)):
                                bxA, bxB = {}, {}
                                sA.append(mk_diag_s(hA, js, bxA))
                                eA.append(mk_diag_e(hA, js, bxA))
                                sB.append(mk_diag_s(hB, js, bxB))
                                eB.append(mk_diag_e(hB, js, bxB))
                        # pipeline: sA0 sB0 | eA0 sA1 eB0 sB1 | eA1 sA2 ...
                        n = len(sA)
                        if n:
                            units.append(sA[0])
                            units.append(sB[0])
                            for i in range(n):
                                if i + 1 < n:
                                    units.append(eA[i])
                                    units.append(sA[i + 1])
                                    units.append(eB[i])
                                    units.append(sB[i + 1])
                                else:
                                    units.append(eA[i])
                                    units.append(eB[i])
                        units.append(mk_norm(hA))
                        units.append(mk_norm(hB))
                    return units

                # ---------------- phase D units ----------------
                def d_units(qb, use_act=False):
                    units = []
                    for qt in range(4):
                        def mk(qt=qt):
                            rows = slice(512 * qb + 128 * qt,
                                         512 * qb + 128 * qt + 128)
                            col = 128 * qt

                            def g():
                                ost = sbC.tile([128, 1024], BF16, name="ost",
                                               tag="ost", bufs=3)
                                split = use_act
                                for nb in range(2):
                                    ps_o = psD.tile([128, 512], F32,
                                                    name="ps_o", tag="ps_d",
                                                    bufs=2)
                                    for c in range(2):
                                        nc.tensor.matmul(
                                            ps_o,
                                            ctxTs[c][qb][:, col:col + 128],
                                            wo_sb[:, c,
                                                  512 * nb:512 * nb + 512],
                                            start=(c == 0), stop=(c == 1))
                                    dst = ost[:, 512 * nb:512 * nb + 512]
                                    if use_act and nb == 0:
                                        nc.scalar.copy(dst, ps_o)
                                    else:
                                        nc.vector.tensor_copy(dst, ps_o)
                                    if split:
                                        nc.sync.dma_start(
                                            out=outp[rows,
                                                     512 * nb:512 * nb + 512],
                                            in_=dst)
                                if not split:
                                    nc.sync.dma_start(out=outp[rows, :],
                                                      in_=ost)
                            return g
                        units.append(mk())
                    return units

                # ---------------- global emission ----------------
                def lab_units(units, pfx):
                    out = []
                    for i, u in enumerate(units):
                        def w(u=u, l=f"{pfx}.{i}"):
                            nc._set_label(l)
                            u()
                        out.append(w)
                    return out

                for u in lab_units(strip_units(0), "strip0"):
                    u()
                for qb in range(NQB):
                    bcu = lab_units(bc_units(qb), f"bc{qb}")
                    fill = []
                    if qb == 0:
                        fill += lab_units(strip_units(1), "strip1")
                    if qb == 1:
                        fill += lab_units(strip_units(2), "strip2")
                    if qb == 2:
                        fill += lab_units(strip_units(3), "strip3")
                        fill += lab_units(d_units(0), "d0")
                    if qb == 3:
                        fill += lab_units(d_units(1), "d1")
                        fill += lab_units(d_units(2), "d2")
                    for u in _interleave(bcu, fill):
                        u()
                for u in lab_units(d_units(3, use_act=True), "d3"):
                    u()

    nc.compile()
    return nc


_NC_CACHE = {}


def _get_nc(causal: bool):
    if causal not in _NC_CACHE:
        _NC_CACHE[causal] = _build_nc(causal)
    return _NC_CACHE[causal]


def _host_consts():
    p = np.zeros((128, 128), np.float32)
    idx = np.arange(0, 128, 2)
    p[idx, idx + 1] = -1.0
    p[idx + 1, idx] = 1.0
    psigT = np.ascontiguousarray(p.T)
    ident = np.eye(128, dtype=np.float32)
    m01n = np.where(np.arange(128)[None, :] >= np.arange(128)[:, None],
                    0.0, NEG).astype(np.float32)
    return psigT, ident, m01n


def _numpy_reference(hidden_states, cos, sin, attention_mask, Wq, Wk, Wv, Wo):
    """Generic-mask fallback, pure numpy port of the reference."""
    GROUPS = H // KVH

    def rope(x, c, s):
        c = c[:, None, :, :]
        s = s[:, None, :, :]
        x1, x2 = x[..., ::2], x[..., 1::2]
        xr = np.stack([x1 * c - x2 * s, x1 * s + x2 * c], axis=-1)
        return xr.reshape(x.shape)

    b, sq, d = hidden_states.shape
    q = (hidden_states @ Wq).reshape(b, sq, H, HD).transpose(0, 2, 1, 3)
    k = (hidden_states @ Wk).reshape(b, sq, KVH, HD).transpose(0, 2, 1, 3)
    v = (hidden_states @ Wv).reshape(b, sq, KVH, HD).transpose(0, 2, 1, 3)
    q = rope(q, cos, sin)
    k = rope(k, cos, sin)
    k = np.repeat(k, GROUPS, axis=1)
    v = np.repeat(v, GROUPS, axis=1)
    out = np.zeros((b, sq, d), np.float32)
    for bi in range(b):
        for hi in range(H):
            sc = (q[bi, hi] @ k[bi, hi].T) * SCALE + attention_mask[0, 0]
            sc = sc - sc.max(axis=-1, keepdims=True)
            e = np.exp(sc)
            pr = e / e.sum(axis=-1, keepdims=True)
            ctx = pr @ v[bi, hi]
            out[bi] += ctx @ Wo[hi * HD:(hi + 1) * HD]
    return out


def kernel(**inputs) -> np.ndarray:
    hs = np.asarray(inputs["hidden_states"], np.float32)
    cos = np.asarray(inputs["cos"], np.float32)
    sin = np.asarray(inputs["sin"], np.float32)
    mask = np.asarray(inputs["attention_mask"], np.float32)
    Wq = np.asarray(inputs["Wq"], np.float32)
    Wk = np.asarray(inputs["Wk"], np.float32)
    Wv = np.asarray(inputs["Wv"], np.float32)
    Wo = np.asarray(inputs["Wo"], np.float32)

    m = mask.reshape(S, S)
    tril = np.tril(np.ones((S, S), dtype=bool))
    causal_ref = np.where(tril, np.float32(0.0), np.float32(NEG))
    if np.array_equal(m, causal_ref):
        causal = True
    elif not m.any():
        causal = False
    else:
        return _numpy_reference(hs, cos, sin, mask, Wq, Wk, Wv, Wo)

    nc = _get_nc(causal)
    psigT, ident, m01 = _host_consts()
    chan_half = (np.arange(64) // 2)
    bf = ml_dtypes.bfloat16

    in_maps = []
    for core in range(8):
        b, t = core // TP, core % TP
        hT = np.ascontiguousarray(hs[b].T).astype(bf)
        hT3 = np.ascontiguousarray(
            hT.reshape(8, 128, S).transpose(1, 0, 2))
        cs_v = np.ascontiguousarray(cos[b].T[chan_half, :]).astype(bf)
        sn_v = np.ascontiguousarray(sin[b].T[chan_half, :]).astype(bf)
        wq_s = Wq[:, t * 256:(t + 1) * 256].astype(bf)
        wq3 = np.ascontiguousarray(wq_s.reshape(8, 128, 256).transpose(1, 0, 2))
        wkv_s = np.concatenate([Wk[:, t * 64:(t + 1) * 64],
                                Wv[:, t * 64:(t + 1) * 64]], axis=1).astype(bf)
        wkv3 = np.ascontiguousarray(
            wkv_s.reshape(8, 128, 128).transpose(1, 0, 2)
            .reshape(128, 4, 256))
        wo_s = Wo[t * 256:(t + 1) * 256]
        # ctxT channel order per chunk: c0 = [h0|h2], c1 = [h1|h3]
        wo_p = np.concatenate([wo_s[0:64], wo_s[128:192],
                               wo_s[64:128], wo_s[192:256]], axis=0).astype(bf)
        wo3 = np.ascontiguousarray(wo_p.reshape(2, 128, D).transpose(1, 0, 2))
        in_maps.append({
            "hT3": hT3, "cs": cs_v, "sn": sn_v,
            "wq3": wq3, "wkv3": wkv3, "wo3": wo3,
            "psigT": psigT.astype(bf), "ident": ident.astype(bf),
            "m01": m01.astype(bf),
            "onesc": np.ones((128, 64), bf),
        })

    res = run_bass_kernel_spmd(nc, in_maps, core_ids=list(range(8)))
    out = np.zeros((B, S, D), np.float32)
    for core in range(8):
        out[core // TP] += np.asarray(res.results[core]["out"],
                                      dtype=np.float32)
    return out


# revision 4
# speedup vs baseline: 1.0120x; 1.0003x over previous
"""Self-contained Trainium2 Bass kernel for GQA MultiHeadAttention with RoPE.

Problem: B=2, S=2048, D=1024, H=16 Q heads, KVH=4 KV heads, head_dim=64,
causal additive mask, f32 in/out.

Sharding: TP=4 over heads (4 Q heads + 1 KV head per shard) x DP=2 over
batch = 8 NeuronCores. Wo sharded on input dim; host sums the 4 partial
outputs per batch element.

Design notes (v2):
- All-bf16 datapath (weights, hidden, q/k/v, exp probs, ctx, out partials);
  psum accumulation stays f32. Error budget 2e-2 rel leaves ~7x headroom.
- Phase A (projections+rope) is emitted strip-wise (512 seq cols) and
  interleaved with attention blocks so PE never idles on exp latency.
- Softmax normalization is fused on-psum: DVE reciprocal of the rowsum row
  (row 64 of the ctx psum, produced by an ones-column in vsm), PE ones-matmul
  broadcast of the recip row, DVE multiply straight out of psum into ctxT.
  No DRAM scratch, no reshape bounces.
- Diagonal (causal-edge) tiles are batched into 2 grouped psum slots per
  (head, qb) so exps are [128, 896]+[128, 384] instead of 4 small ones.
- DMAs are coalesced (one per hT strip, one out-store per 128-row tile) and
  issued on the engine that produced their source.
"""

import os
import sys

for _p in ("/opt/trn_rl_repo", "/root/.axon_site/_ro/trn_rl_repo"):
    if os.path.isdir(_p) and _p not in sys.path:
        sys.path.insert(0, _p)

import numpy as np
import ml_dtypes

import concourse.bacc as bacc
import concourse.bass as bass
import concourse.tile as tile
from concourse import mybir
from concourse.bass_utils import run_bass_kernel_spmd

F32 = mybir.dt.float32
F32R = mybir.dt.float32r
BF16 = mybir.dt.bfloat16
AF = mybir.ActivationFunctionType

H, KVH, HD = 16, 4, 64
B, S, D = 2, 2048, 1024
TP = 4
SCALE = HD ** -0.5
NEG = -1e9
NT = S // 128               # 16 kv tiles
NQB = S // 512              # 4 q blocks


def _interleave(a, b):
    """Merge two unit lists proportionally (a is the primary stream)."""
    if not b:
        return list(a)
    if not a:
        return list(b)
    out = []
    na, nb = len(a), len(b)
    ia = ib = 0
    while ia < na or ib < nb:
        # emit whichever stream is behind proportionally
        if ib >= nb or (ia < na and ia * nb <= ib * na):
            out.append(a[ia]); ia += 1
        else:
            out.append(b[ib]); ib += 1
    return out


MM_LABELS = []


def _build_nc(causal: bool):
    nc = bacc.Bacc()
    MM_LABELS.clear()
    _orig_mm = nc.tensor.matmul
    _cur = {"l": "?"}

    def _mm(*a, **kw):
        MM_LABELS.append(_cur["l"])
        return _orig_mm(*a, **kw)
    nc.tensor.matmul = _mm

    def _lab(s):
        _cur["l"] = s
    nc._set_label = _lab

    hT3 = nc.declare_dram_parameter("hT3", [128, 8, S], BF16, isOutput=False)
    cs = nc.declare_dram_parameter("cs", [64, S], BF16, isOutput=False)
    sn = nc.declare_dram_parameter("sn", [64, S], BF16, isOutput=False)
    wq3 = nc.declare_dram_parameter("wq3", [128, 8, 256], BF16, isOutput=False)
    wkv3 = nc.declare_dram_parameter("wkv3", [128, 4, 256], BF16, isOutput=False)
    wo3 = nc.declare_dram_parameter("wo3", [128, 2, D], BF16, isOutput=False)
    psigT = nc.declare_dram_parameter("psigT", [128, 128], BF16, isOutput=False)
    ident = nc.declare_dram_parameter("ident", [128, 128], BF16, isOutput=False)
    m01 = nc.declare_dram_parameter("m01", [128, 128], BF16, isOutput=False)
    onesc = nc.declare_dram_parameter("onesc", [128, 64], BF16, isOutput=False)
    outp = nc.declare_dram_parameter("out", [S, D], BF16, isOutput=True)

    with tile.TileContext(nc) as tc, nc.allow_low_precision(
            reason="2e-2 rel tolerance; bf16 throughout"):
        with tc.tile_pool(name="hold", bufs=1) as hp:
            wkv_sb = hp.tile([128, 4, 256], BF16, name="wkv_sb", tag="wkv_sb")
            psig_sb = hp.tile([128, 128], BF16, name="psig_sb", tag="psig_sb")
            cosf = hp.tile([128, S], BF16, name="cosf", tag="cosf")
            sinf = hp.tile([128, S], BF16, name="sinf", tag="sinf")
            wq_sb = hp.tile([128, 8, 256], BF16, name="wq_sb", tag="wq_sb")
            id_sb = hp.tile([128, 128], BF16, name="id_sb", tag="id_sb")
            m01_sb = hp.tile([128, 128], BF16, name="m01_sb", tag="m01_sb")
            ones_sb = hp.tile([128, 64], BF16, name="ones_sb", tag="ones_sb")
            wo_sb = hp.tile([128, 2, D], BF16, name="wo_sb", tag="wo_sb")
            ht_sb = hp.tile([128, 8, S], BF16, name="ht_sb", tag="ht_sb")
            qTs = [hp.tile([128, S], BF16, name=f"qT{p}", tag=f"qT{p}")
                   for p in range(2)]
            kT = hp.tile([128, S], BF16, name="kTt", tag="kTt")
            vsm = hp.tile([128, NT, 65], BF16, name="vsm", tag="vsm")
            ctxTs = [[hp.tile([128, 512], BF16, name=f"ctxT{c}_{q}",
                              tag=f"ctxT{c}_{q}") for q in range(NQB)]
                     for c in range(2)]

            # ---- prologue DMAs (SP queue), ordered by first use ----
            nc.sync.dma_start(out=wkv_sb, in_=wkv3[:, :, :])
            nc.sync.dma_start(out=ht_sb[:, 0:2, 0:512], in_=hT3[:, 0:2, 0:512])
            nc.sync.dma_start(out=ht_sb[:, 2:4, 0:512], in_=hT3[:, 2:4, 0:512])
            nc.sync.dma_start(out=ht_sb[:, 4:6, 0:512], in_=hT3[:, 4:6, 0:512])
            nc.sync.dma_start(out=ht_sb[:, 6:8, 0:512], in_=hT3[:, 6:8, 0:512])
            nc.sync.dma_start(out=psig_sb, in_=psigT[:, :])
            nc.sync.dma_start(out=cosf[0:64, :], in_=cs[:, :])
            nc.sync.dma_start(out=cosf[64:128, :], in_=cs[:, :])
            nc.sync.dma_start(out=sinf[0:64, :], in_=sn[:, :])
            nc.sync.dma_start(out=sinf[64:128, :], in_=sn[:, :])
            nc.sync.dma_start(out=wq_sb, in_=wq3[:, :, :])
            nc.sync.dma_start(out=ht_sb[:, :, 512:1024], in_=hT3[:, :, 512:1024])
            nc.sync.dma_start(out=id_sb, in_=ident[:, :])
            nc.sync.dma_start(out=m01_sb, in_=m01[:, :])
            nc.sync.dma_start(out=ones_sb, in_=onesc[:, :])
            nc.sync.dma_start(out=wo_sb, in_=wo3[:, :, :])
            nc.sync.dma_start(out=ht_sb[:, :, 1024:1536], in_=hT3[:, :, 1024:1536])
            nc.sync.dma_start(out=ht_sb[:, :, 1536:2048], in_=hT3[:, :, 1536:2048])
            # ones column of vsm for the softmax denominator row
            nc.gpsimd.memset(vsm[:, :, 64:65], 1.0)
            # warm the ACT exp table while ACT is idle (the implicit load
            # would otherwise land on the first attention exp)
            actw = hp.tile([1, 16], F32, name="actw", tag="actw")
            nc.gpsimd.memset(actw[:, :], 1.0)
            nc.scalar.activation(actw, actw, AF.Exp, scale=1.0)

            with tc.tile_pool(name="psS", bufs=1, space="PSUM") as psS, \
                 tc.tile_pool(name="psD", bufs=1, space="PSUM") as psD, \
                 tc.tile_pool(name="psC", bufs=1, space="PSUM") as psC, \
                 tc.tile_pool(name="etp", bufs=1) as etp, \
                 tc.tile_pool(name="sbA", bufs=1) as sbA, \
                 tc.tile_pool(name="sbC", bufs=1) as sbC:

                # ---------------- phase A units (per strip) ----------------
                def u_kv(sc):
                    csl = slice(512 * sc, 512 * sc + 512)

                    st = {}

                    def f1():
                        ps = psD.tile([128, 512], F32, name="ps_kv",
                                      tag="ps_d", bufs=2)
                        for dc in range(4):
                            nc.tensor.matmul(
                                ps, wkv_sb[:, dc // 2,
                                           128 * (dc % 2):128 * (dc % 2) + 128],
                                ht_sb[:, dc, csl],
                                start=(dc == 0), stop=False)
                        st["ps"] = ps

                    def f2():
                        ps = st["ps"]
                        for dc in range(4, 8):
                            nc.tensor.matmul(
                                ps, wkv_sb[:, dc // 2,
                                           128 * (dc % 2):128 * (dc % 2) + 128],
                                ht_sb[:, dc, csl],
                                start=False, stop=(dc == 7))
                        kvraw = sbA.tile([128, 512], BF16, name="kvraw",
                                         tag="kvraw", bufs=3)
                        nc.vector.tensor_copy(kvraw, ps)
                        return kvraw
                    return f1, f2

                def u_kv_rope(sc, get):
                    csl = slice(512 * sc, 512 * sc + 512)

                    def f():
                        kvraw = get()
                        ps_kr = psD.tile([128, 512], F32, name="ps_kr",
                                         tag="ps_d", bufs=2)[0:64, :]
                        nc.tensor.matmul(ps_kr, psig_sb[0:64, 0:64],
                                         kvraw[0:64, :], start=True, stop=True)
                        kdst = kT[0:64, csl]
                        nc.gpsimd.tensor_mul(kdst, kvraw[0:64, :],
                                             cosf[0:64, csl])
                        ktmp = sbA.tile([64, 512], BF16, name="ktmp",
                                        tag="ktmp", bufs=2)
                        nc.vector.tensor_mul(ktmp, ps_kr, sinf[0:64, csl])
                        nc.gpsimd.tensor_add(kdst, kdst, ktmp)
                        # duplicate roped K to partitions 64:128 (odd heads)
                        nc.sync.dma_start(out=kT[64:128, csl], in_=kdst)
                    return f

                def u_v_t(sc, get):
                    def f():
                        kvraw = get()
                        vst = sbA.tile([128, 4, 64], BF16, name="vst",
                                       tag="vst", bufs=2)
                        nc.sync.dma_start_transpose(
                            out=vst[:, :, :], in_=kvraw[64:128, :])
                        nc.vector.tensor_copy(vsm[:, 4 * sc:4 * sc + 4, 0:64],
                                              vst[:, :, :])
                    return f

                def u_q(sc, pp):
                    csl = slice(512 * sc, 512 * sc + 512)

                    st = {}

                    def f1():
                        ps = psD.tile([128, 512], F32, name="ps_q",
                                      tag="ps_d", bufs=2)
                        for dc in range(4):
                            nc.tensor.matmul(
                                ps, wq_sb[:, dc, 128 * pp:128 * pp + 128],
                                ht_sb[:, dc, csl],
                                start=(dc == 0), stop=False)
                        st["ps"] = ps

                    def f2():
                        ps = st["ps"]
                        for dc in range(4, 8):
                            nc.tensor.matmul(
                                ps, wq_sb[:, dc, 128 * pp:128 * pp + 128],
                                ht_sb[:, dc, csl],
                                start=False, stop=(dc == 7))
                        qraw = sbA.tile([128, 512], BF16, name="qraw",
                                        tag="qraw", bufs=3)
                        nc.vector.tensor_copy(qraw, ps)
                        return qraw
                    return f1, f2

                def u_q_rope(sc, pp, get):
                    csl = slice(512 * sc, 512 * sc + 512)

                    def f():
                        qraw = get()
                        ps_rot = psD.tile([128, 512], F32, name="ps_rot",
                                          tag="ps_d", bufs=2)
                        nc.tensor.matmul(ps_rot, psig_sb, qraw,
                                         start=True, stop=True)
                        dst = qTs[pp][:, csl]
                        nc.gpsimd.tensor_mul(dst, qraw, cosf[:, csl])
                        rtmp = sbA.tile([128, 512], BF16, name="rtmp",
                                        tag="rtmp", bufs=3)
                        nc.vector.tensor_mul(rtmp, ps_rot, sinf[:, csl])
                        nc.gpsimd.tensor_add(dst, dst, rtmp)
                    return f

                def strip_units(sc):
                    box = {}

                    def mk(key, fn):
                        def g():
                            box[key] = fn()
                        return g

                    def rd(key):
                        return lambda: box[key]

                    kv1, kv2 = u_kv(sc)
                    q01, q02 = u_q(sc, 0)
                    q11, q12 = u_q(sc, 1)
                    return [
                        kv1,
                        mk("kv", kv2),
                        q01,
                        mk("q0", q02),
                        u_kv_rope(sc, rd("kv")),
                        u_q_rope(sc, 0, rd("q0")),
                        u_v_t(sc, rd("kv")),
                        q11,
                        mk("q1", q12),
                        u_q_rope(sc, 1, rd("q1")),
                    ]

                # ---------------- attention units ----------------
                # per (qb, sp): heads hA=2sp, hB=2sp+1 interleaved
                def bc_units(qb, only_sp=None):
                    qsl = slice(512 * qb, 512 * qb + 512)
                    units = []
                    sps = (1, 0) if qb == NQB - 1 else (0, 1)
                    if only_sp is not None:
                        sps = (only_sp,)
                    for sp in sps:
                        hs = [2 * sp, 2 * sp + 1]
                        ctxps = {}
                        nki = (4 * qb + 4) if causal else NT
                        nfull = (4 * qb) if causal else NT

                        def mk_ctx_alloc(h):
                            def g():
                                ctxps[h] = psC.tile([128, 512], F32,
                                                    name="ps_ctx",
                                                    tag="ps_ctx", bufs=2)
                            return g

                        # scores+exp+ctx closures
                        def mk_pair_s(h, kp, bx):
                            off = 64 * (h % 2)
                            pp = h // 2

                            def g():
                                ps = psS.tile([128, 1024], F32, name="ps_s",
                                              tag="ps_s", bufs=2)
                                for jj in range(2):
                                    ki = kp + jj
                                    nc.tensor.matmul(
                                        ps[:, 512 * jj:512 * jj + 512],
                                        kT[off:off + 64,
                                           128 * ki:128 * ki + 128],
                                        qTs[pp][off:off + 64, qsl],
                                        start=True, stop=True)
                                bx["ps"] = ps
                            return g

                        def mk_pair_e(h, kp, bx, _nki=nki):
                            def g():
                                et = etp.tile([128, 1024], BF16, name="et",
                                              tag="et", bufs=6)
                                nc.scalar.activation(et, bx["ps"], AF.Exp,
                                                     scale=SCALE)
                                for jj in range(2):
                                    ki = kp + jj
                                    nc.tensor.matmul(
                                        ctxps[h][0:65, :],
                                        vsm[:, ki, 0:65],
                                        et[:, 512 * jj:512 * jj + 512],
                                        start=(ki == 0),
                                        stop=(ki == _nki - 1))
                            return g

                        # diag group: js = (0,1) or (2,3); slot offsets
                        def mk_diag_s(h, js, bx):
                            off = 64 * (h % 2)
                            pp = h // 2

                            def g():
                                spans = [512 - 128 * j for j in js]
                                tot = sum(spans)
                                ps = psS.tile([128, 1024], F32, name="ps_g",
                                              tag="ps_s", bufs=2)
                                o = 0
                                for j, span in zip(js, spans):
                                    ki = 4 * qb + j
                                    # diagonal 128-col block: scores then the
                                    # folded causal mask (+= I^T @ -1e9 tri)
                                    nc.tensor.matmul(
                                        ps[:, o:o + 128],
                                        kT[off:off + 64,
                                           128 * ki:128 * ki + 128],
                                        qTs[pp][off:off + 64,
                                                512 * qb + 128 * j:
                                                512 * qb + 128 * j + 128],
                                        start=True, stop=False)
                                    nc.tensor.matmul(
                                        ps[:, o:o + 128], id_sb, m01_sb,
                                        start=False, stop=True)
                                    if span > 128:
                                        # fully-causal remainder of the span
                                        nc.tensor.matmul(
                                            ps[:, o + 128:o + span],
                                            kT[off:off + 64,
                                               128 * ki:128 * ki + 128],
                                            qTs[pp][off:off + 64,
                                                    512 * qb + 128 * (j + 1):
                                                    512 * qb + 512],
                                            start=True, stop=True)
                                    o += span
                                bx["ps"] = ps
                                bx["tot"] = tot
                            return g

                        def mk_diag_e(h, js, bx, _nki=nki):
                            def g():
                                spans = [512 - 128 * j for j in js]
                                et = etp.tile([128, 1024], BF16, name="etd",
                                              tag="et", bufs=6)
                                nc.scalar.activation(et[:, 0:bx["tot"]],
                                                     bx["ps"][:, 0:bx["tot"]],
                                                     AF.Exp, scale=SCALE)
                                o = 0
                                for j, span in zip(js, spans):
                                    ki = 4 * qb + j
                                    nc.tensor.matmul(
                                        ctxps[h][0:65, 128 * j:512],
                                        vsm[:, ki, 0:65],
                                        et[:, o:o + span],
                                        start=(ki == 0),
                                        stop=(ki == _nki - 1))
                                    o += span
                            return g

                        def mk_norm(h):
                            c = h % 2
                            up = h // 2   # 0: partitions 0:64, 1: 64:128

                            def g():
                                pc = ctxps[h]
                                ctxu = sbC.tile([65, 512], BF16, name="ctxu",
                                                tag="ctxu", bufs=4)
                                nc.vector.tensor_copy(ctxu, pc[0:65, :])
                                nc.vector.reciprocal(ctxu[64:65, :],
                                                     ctxu[64:65, :])
                                ps_b = psD.tile([128, 512], F32, name="ps_b",
                                                tag="ps_d", bufs=2)[0:64, :]
                                nc.tensor.matmul(
                                    ps_b, ones_sb[64:65, :],
                                    ctxu[64:65, :],
                                    start=True, stop=True)
                                if up == 0:
                                    nc.vector.tensor_mul(
                                        ctxTs[c][qb][0:64, :],
                                        ctxu[0:64, :], ps_b)
                                else:
                                    ctmp = sbC.tile([64, 512], BF16,
                                                    name="ctmp", tag="ctmp",
                                                    bufs=2)
                                    nc.vector.tensor_mul(ctmp, ctxu[0:64, :],
                                                         ps_b)
                                    nc.sync.dma_start(
                                        out=ctxTs[c][qb][64:128, :], in_=ctmp)
                            return g

                        # build interleaved 2-head stream for this sp
                        hA, hB = hs
                        units.append(mk_ctx_alloc(hA))
                        units.append(mk_ctx_alloc(hB))
                        boxes = {h: [] for h in hs}
                        sA, eA, sB, eB = [], [], [], []
                        for kp in range(0, nfull, 2):
                            bxA, bxB = {}, {}
                            sA.append(mk_pair_s(hA, kp, bxA))
                            eA.append(mk_pair_e(hA, kp, bxA))
                            sB.append(mk_pair_s(hB, kp, bxB))
                            eB.append(mk_pair_e(hB, kp, bxB))
                        if causal:
                            for js in ((0, 1), (2, 3)):
                                bxA, bxB = {}, {}
                                sA.append(mk_diag_s(hA, js, bxA))
                                eA.append(mk_diag_e(hA, js, bxA))
                                sB.append(mk_diag_s(hB, js, bxB))
                                eB.append(mk_diag_e(hB, js, bxB))
                        # pipeline: sA0 sB0 | eA0 sA1 eB0 sB1 | eA1 sA2 ...
                        n = len(sA)
                        if n:
                            units.append(sA[0])
                            units.append(sB[0])
                            for i in range(n):
                                if i + 1 < n:
                                    units.append(eA[i])
                                    units.append(sA[i + 1])
                                    units.append(eB[i])
                                    units.append(sB[i + 1])
                                else:
                                    units.append(eA[i])
                                    units.append(eB[i])
                        units.append(mk_norm(hA))
                        units.append(mk_norm(hB))
                    return units

                # ---------------- phase D units ----------------
                def d_units(qb, use_act=False):
                    units = []
                    for qt in range(4):
                        def mk(qt=qt):
                            rows = slice(512 * qb + 128 * qt,
                                         512 * qb + 128 * qt + 128)
                            col = 128 * qt

                            def g():
                                ost = sbC.tile([128, 1024], BF16, name="ost",
                                               tag="ost", bufs=3)
                                split = use_act
                                for nb in range(2):
                                    ps_o = psD.tile([128, 512], F32,
                                                    name="ps_o", tag="ps_d",
                                                    bufs=2)
                                    for c in range(2):
                                        nc.tensor.matmul(
                                            ps_o,
                                            ctxTs[c][qb][:, col:col + 128],
                                            wo_sb[:, c,
                                                  512 * nb:512 * nb + 512],
                                            start=(c == 0), stop=(c == 1))
                                    dst = ost[:, 512 * nb:512 * nb + 512]
                                    if use_act and nb == 0:
                                        nc.scalar.copy(dst, ps_o)
                                    else:
                                        nc.vector.tensor_copy(dst, ps_o)
                                    if split:
                                        nc.sync.dma_start(
                                            out=outp[rows,
                                                     512 * nb:512 * nb + 512],
                                            in_=dst)
                                if not split:
                                    nc.sync.dma_start(out=outp[rows, :],
                                                      in_=ost)
                            return g
                        units.append(mk())
                    return units

                # ---------------- global emission ----------------
                def lab_units(units, pfx):
                    out = []
                    for i, u in enumerate(units):
                        def w(u=u, l=f"{pfx}.{i}"):
                            nc._set_label(l)
                            u()
                        out.append(w)
                    return out

                for u in lab_units(strip_units(0), "strip0"):
                    u()
                for qb in range(NQB):
                    bcu = lab_units(bc_units(qb), f"bc{qb}")
                    fill = []
                    if qb == 0:
                        fill += lab_units(strip_units(1), "strip1")
                    if qb == 1:
                        fill += lab_units(strip_units(2), "strip2")
                    if qb == 2:
                        fill += lab_units(strip_units(3), "strip3")
                    if qb == 3:
                        fill += lab_units(d_units(0), "d0")
                        fill += lab_units(d_units(1), "d1")
                        fill += lab_units(d_units(2), "d2")
                    for u in _interleave(bcu, fill):
                        u()
                for u in lab_units(d_units(3, use_act=True), "d3"):
                    u()

    nc.compile()
    return nc


_NC_CACHE = {}


def _get_nc(causal: bool):
    if causal not in _NC_CACHE:
        _NC_CACHE[causal] = _build_nc(causal)
    return _NC_CACHE[causal]


def _host_consts():
    p = np.zeros((128, 128), np.float32)
    idx = np.arange(0, 128, 2)
    p[idx, idx + 1] = -1.0
    p[idx + 1, idx] = 1.0
    psigT = np.ascontiguousarray(p.T)
    ident = np.eye(128, dtype=np.float32)
    m01n = np.where(np.arange(128)[None, :] >= np.arange(128)[:, None],
                    0.0, NEG).astype(np.float32)
    return psigT, ident, m01n


def _numpy_reference(hidden_states, cos, sin, attention_mask, Wq, Wk, Wv, Wo):
    """Generic-mask fallback, pure numpy port of the reference."""
    GROUPS = H // KVH

    def rope(x, c, s):
        c = c[:, None, :, :]
        s = s[:, None, :, :]
        x1, x2 = x[..., ::2], x[..., 1::2]
        xr = np.stack([x1 * c - x2 * s, x1 * s + x2 * c], axis=-1)
        return xr.reshape(x.shape)

    b, sq, d = hidden_states.shape
    q = (hidden_states @ Wq).reshape(b, sq, H, HD).transpose(0, 2, 1, 3)
    k = (hidden_states @ Wk).reshape(b, sq, KVH, HD).transpose(0, 2, 1, 3)
    v = (hidden_states @ Wv).reshape(b, sq, KVH, HD).transpose(0, 2, 1, 3)
    q = rope(q, cos, sin)
    k = rope(k, cos, sin)
    k = np.repeat(k, GROUPS, axis=1)
    v = np.repeat(v, GROUPS, axis=1)
    out = np.zeros((b, sq, d), np.float32)
    for bi in range(b):
        for hi in range(H):
            sc = (q[bi, hi] @ k[bi, hi].T) * SCALE + attention_mask[0, 0]
            sc = sc - sc.max(axis=-1, keepdims=True)
            e = np.exp(sc)
            pr = e / e.sum(axis=-1, keepdims=True)
            ctx = pr @ v[bi, hi]
            out[bi] += ctx @ Wo[hi * HD:(hi + 1) * HD]
    return out


def kernel(**inputs) -> np.ndarray:
    hs = np.asarray(inputs["hidden_states"], np.float32)
    cos = np.asarray(inputs["cos"], np.float32)
    sin = np.asarray(inputs["sin"], np.float32)
    mask = np.asarray(inputs["attention_mask"], np.float32)
    Wq = np.asarray(inputs["Wq"], np.float32)
    Wk = np.asarray(inputs["Wk"], np.float32)
    Wv = np.asarray(inputs["Wv"], np.float32)
    Wo = np.asarray(inputs["Wo"], np.float32)

    m = mask.reshape(S, S)
    tril = np.tril(np.ones((S, S), dtype=bool))
    causal_ref = np.where(tril, np.float32(0.0), np.float32(NEG))
    if np.array_equal(m, causal_ref):
        causal = True
    elif not m.any():
        causal = False
    else:
        return _numpy_reference(hs, cos, sin, mask, Wq, Wk, Wv, Wo)

    nc = _get_nc(causal)
    psigT, ident, m01 = _host_consts()
    chan_half = (np.arange(64) // 2)
    bf = ml_dtypes.bfloat16

    in_maps = []
    for core in range(8):
        b, t = core // TP, core % TP
        hT = np.ascontiguousarray(hs[b].T).astype(bf)
        hT3 = np.ascontiguousarray(
            hT.reshape(8, 128, S).transpose(1, 0, 2))
        cs_v = np.ascontiguousarray(cos[b].T[chan_half, :]).astype(bf)
        sn_v = np.ascontiguousarray(sin[b].T[chan_half, :]).astype(bf)
        wq_s = Wq[:, t * 256:(t + 1) * 256].astype(bf)
        wq3 = np.ascontiguousarray(wq_s.reshape(8, 128, 256).transpose(1, 0, 2))
        wkv_s = np.concatenate([Wk[:, t * 64:(t + 1) * 64],
                                Wv[:, t * 64:(t + 1) * 64]], axis=1).astype(bf)
        wkv3 = np.ascontiguousarray(
            wkv_s.reshape(8, 128, 128).transpose(1, 0, 2)
            .reshape(128, 4, 256))
        wo_s = Wo[t * 256:(t + 1) * 256]
        # ctxT channel order per chunk: c0 = [h0|h2], c1 = [h1|h3]
        wo_p = np.concatenate([wo_s[0:64], wo_s[128:192],
                               wo_s[64:128], wo_s[192:256]], axis=0).astype(bf)
        wo3 = np.ascontiguousarray(wo_p.reshape(2, 128, D).transpose(1, 0, 2))
        in_maps.append({
            "hT3": hT3, "cs": cs_v, "sn": sn_v,
            "wq3": wq3, "wkv3": wkv3, "wo3": wo3,
            "psigT": psigT.astype(bf), "ident": ident.astype(bf),
            "m01": m01.astype(bf),
            "onesc": np.ones((128, 64), bf),
        })

    res = run_bass_kernel_spmd(nc, in_maps, core_ids=list(range(8)))
    out = np.zeros((B, S, D), np.float32)
    for core in range(8):
        out[core // TP] += np.asarray(res.results[core]["out"],
                                      dtype=np.float32)
    return out


# revision 5
# speedup vs baseline: 1.0147x; 1.0028x over previous
"""Self-contained Trainium2 Bass kernel for GQA MultiHeadAttention with RoPE.

Problem: B=2, S=2048, D=1024, H=16 Q heads, KVH=4 KV heads, head_dim=64,
causal additive mask, f32 in/out.

Sharding: TP=4 over heads (4 Q heads + 1 KV head per shard) x DP=2 over
batch = 8 NeuronCores. Wo sharded on input dim; host sums the 4 partial
outputs per batch element.

Design notes (v2):
- All-bf16 datapath (weights, hidden, q/k/v, exp probs, ctx, out partials);
  psum accumulation stays f32. Error budget 2e-2 rel leaves ~7x headroom.
- Phase A (projections+rope) is emitted strip-wise (512 seq cols) and
  interleaved with attention blocks so PE never idles on exp latency.
- Softmax normalization is fused on-psum: DVE reciprocal of the rowsum row
  (row 64 of the ctx psum, produced by an ones-column in vsm), PE ones-matmul
  broadcast of the recip row, DVE multiply straight out of psum into ctxT.
  No DRAM scratch, no reshape bounces.
- Diagonal (causal-edge) tiles are batched into 2 grouped psum slots per
  (head, qb) so exps are [128, 896]+[128, 384] instead of 4 small ones.
- DMAs are coalesced (one per hT strip, one out-store per 128-row tile) and
  issued on the engine that produced their source.
"""

import os
import sys

for _p in ("/opt/trn_rl_repo", "/root/.axon_site/_ro/trn_rl_repo"):
    if os.path.isdir(_p) and _p not in sys.path:
        sys.path.insert(0, _p)

import numpy as np
import ml_dtypes

import concourse.bacc as bacc
import concourse.bass as bass
import concourse.tile as tile
from concourse import mybir
from concourse.bass_utils import run_bass_kernel_spmd

F32 = mybir.dt.float32
F32R = mybir.dt.float32r
BF16 = mybir.dt.bfloat16
AF = mybir.ActivationFunctionType

H, KVH, HD = 16, 4, 64
B, S, D = 2, 2048, 1024
TP = 4
SCALE = HD ** -0.5
NEG = -1e9
NT = S // 128               # 16 kv tiles
NQB = S // 512              # 4 q blocks


def _interleave(a, b):
    """Merge two unit lists proportionally (a is the primary stream)."""
    if not b:
        return list(a)
    if not a:
        return list(b)
    out = []
    na, nb = len(a), len(b)
    ia = ib = 0
    while ia < na or ib < nb:
        # emit whichever stream is behind proportionally
        if ib >= nb or (ia < na and ia * nb <= ib * na):
            out.append(a[ia]); ia += 1
        else:
            out.append(b[ib]); ib += 1
    return out


MM_LABELS = []


def _build_nc(causal: bool):
    nc = bacc.Bacc()
    MM_LABELS.clear()
    _orig_mm = nc.tensor.matmul
    _cur = {"l": "?"}

    def _mm(*a, **kw):
        MM_LABELS.append(_cur["l"])
        return _orig_mm(*a, **kw)
    nc.tensor.matmul = _mm

    def _lab(s):
        _cur["l"] = s
    nc._set_label = _lab

    hT3 = nc.declare_dram_parameter("hT3", [128, 8, S], BF16, isOutput=False)
    cs = nc.declare_dram_parameter("cs", [64, S], BF16, isOutput=False)
    sn = nc.declare_dram_parameter("sn", [64, S], BF16, isOutput=False)
    wqA = nc.declare_dram_parameter("wqA", [128, 4, 256], BF16, isOutput=False)
    wqB = nc.declare_dram_parameter("wqB", [128, 4, 256], BF16, isOutput=False)
    wkv3 = nc.declare_dram_parameter("wkv3", [128, 4, 256], BF16, isOutput=False)
    wo3 = nc.declare_dram_parameter("wo3", [128, 2, D], BF16, isOutput=False)
    psigT = nc.declare_dram_parameter("psigT", [128, 128], BF16, isOutput=False)
    ident = nc.declare_dram_parameter("ident", [128, 128], BF16, isOutput=False)
    m01 = nc.declare_dram_parameter("m01", [128, 128], BF16, isOutput=False)
    onesc = nc.declare_dram_parameter("onesc", [128, 64], BF16, isOutput=False)
    outp = nc.declare_dram_parameter("out", [S, D], BF16, isOutput=True)

    with tile.TileContext(nc) as tc, nc.allow_low_precision(
            reason="2e-2 rel tolerance; bf16 throughout"):
        with tc.tile_pool(name="hold", bufs=1) as hp:
            wkv_sb = hp.tile([128, 4, 256], BF16, name="wkv_sb", tag="wkv_sb")
            psig_sb = hp.tile([128, 128], BF16, name="psig_sb", tag="psig_sb")
            cosf = hp.tile([128, S], BF16, name="cosf", tag="cosf")
            sinf = hp.tile([128, S], BF16, name="sinf", tag="sinf")
            wq_sbs = [hp.tile([128, 4, 256], BF16, name=f"wq_sb{i}",
                              tag=f"wq_sb{i}") for i in range(2)]
            id_sb = hp.tile([128, 128], BF16, name="id_sb", tag="id_sb")
            m01_sb = hp.tile([128, 128], BF16, name="m01_sb", tag="m01_sb")
            ones_sb = hp.tile([128, 64], BF16, name="ones_sb", tag="ones_sb")
            wo_sb = hp.tile([128, 2, D], BF16, name="wo_sb", tag="wo_sb")
            ht_sb = hp.tile([128, 8, S], BF16, name="ht_sb", tag="ht_sb")
            qTs = [hp.tile([128, S], BF16, name=f"qT{p}", tag=f"qT{p}")
                   for p in range(2)]
            kT = hp.tile([128, S], BF16, name="kTt", tag="kTt")
            vsm = hp.tile([128, NT, 65], BF16, name="vsm", tag="vsm")
            ctxTs = [[hp.tile([128, 512], BF16, name=f"ctxT{c}_{q}",
                              tag=f"ctxT{c}_{q}") for q in range(NQB)]
                     for c in range(2)]

            # ---- prologue DMAs (SP queue), ordered by first use ----
            nc.sync.dma_start(out=wkv_sb, in_=wkv3[:, :, :])
            nc.sync.dma_start(out=ht_sb[:, 0:2, 0:512], in_=hT3[:, 0:2, 0:512])
            nc.sync.dma_start(out=ht_sb[:, 2:4, 0:512], in_=hT3[:, 2:4, 0:512])
            nc.sync.dma_start(out=ht_sb[:, 4:6, 0:512], in_=hT3[:, 4:6, 0:512])
            nc.sync.dma_start(out=ht_sb[:, 6:8, 0:512], in_=hT3[:, 6:8, 0:512])
            nc.sync.dma_start(out=psig_sb, in_=psigT[:, :])
            nc.sync.dma_start(out=cosf[0:64, :], in_=cs[:, :])
            nc.sync.dma_start(out=cosf[64:128, :], in_=cs[:, :])
            nc.sync.dma_start(out=sinf[0:64, :], in_=sn[:, :])
            nc.sync.dma_start(out=sinf[64:128, :], in_=sn[:, :])
            nc.sync.dma_start(out=wq_sbs[0], in_=wqA[:, :, :])
            nc.sync.dma_start(out=wq_sbs[1], in_=wqB[:, :, :])
            nc.sync.dma_start(out=ht_sb[:, :, 512:1024], in_=hT3[:, :, 512:1024])
            nc.sync.dma_start(out=id_sb, in_=ident[:, :])
            nc.sync.dma_start(out=m01_sb, in_=m01[:, :])
            nc.sync.dma_start(out=ones_sb, in_=onesc[:, :])
            nc.sync.dma_start(out=wo_sb, in_=wo3[:, :, :])
            nc.sync.dma_start(out=ht_sb[:, :, 1024:1536], in_=hT3[:, :, 1024:1536])
            nc.sync.dma_start(out=ht_sb[:, :, 1536:2048], in_=hT3[:, :, 1536:2048])
            # ones column of vsm for the softmax denominator row
            nc.gpsimd.memset(vsm[:, :, 64:65], 1.0)
            # warm the ACT exp table while ACT is idle (the implicit load
            # would otherwise land on the first attention exp)
            actw = hp.tile([1, 16], F32, name="actw", tag="actw")
            nc.gpsimd.memset(actw[:, :], 1.0)
            nc.scalar.activation(actw, actw, AF.Exp, scale=1.0)


            with tc.tile_pool(name="psS", bufs=1, space="PSUM") as psS, \
                 tc.tile_pool(name="psD", bufs=1, space="PSUM") as psD, \
                 tc.tile_pool(name="psC", bufs=1, space="PSUM") as psC, \
                 tc.tile_pool(name="etp", bufs=1) as etp, \
                 tc.tile_pool(name="sbA", bufs=1) as sbA, \
                 tc.tile_pool(name="sbC", bufs=1) as sbC:

                # ---------------- phase A units (per strip) ----------------
                def u_kv(sc):
                    csl = slice(512 * sc, 512 * sc + 512)

                    st = {}

                    def f1():
                        ps = psD.tile([128, 512], F32, name="ps_kv",
                                      tag="ps_d", bufs=2)
                        for dc in range(4):
                            nc.tensor.matmul(
                                ps, wkv_sb[:, dc // 2,
                                           128 * (dc % 2):128 * (dc % 2) + 128],
                                ht_sb[:, dc, csl],
                                start=(dc == 0), stop=False)
                        st["ps"] = ps

                    def f2():
                        ps = st["ps"]
                        for dc in range(4, 8):
                            nc.tensor.matmul(
                                ps, wkv_sb[:, dc // 2,
                                           128 * (dc % 2):128 * (dc % 2) + 128],
                                ht_sb[:, dc, csl],
                                start=False, stop=(dc == 7))
                        kvraw = sbA.tile([128, 512], BF16, name="kvraw",
                                         tag="kvraw", bufs=3)
                        nc.vector.tensor_copy(kvraw, ps)
                        return kvraw
                    return f1, f2

                def u_kv_rope(sc, get):
                    csl = slice(512 * sc, 512 * sc + 512)

                    def f():
                        kvraw = get()
                        ps_kr = psD.tile([128, 512], F32, name="ps_kr",
                                         tag="ps_d", bufs=2)[0:64, :]
                        nc.tensor.matmul(ps_kr, psig_sb[0:64, 0:64],
                                         kvraw[0:64, :], start=True, stop=True)
                        kdst = kT[0:64, csl]
                        nc.gpsimd.tensor_mul(kdst, kvraw[0:64, :],
                                             cosf[0:64, csl])
                        ktmp = sbA.tile([64, 512], BF16, name="ktmp",
                                        tag="ktmp", bufs=2)
                        nc.vector.tensor_mul(ktmp, ps_kr, sinf[0:64, csl])
                        nc.gpsimd.tensor_add(kdst, kdst, ktmp)
                        # duplicate roped K to partitions 64:128 (odd heads)
                        nc.sync.dma_start(out=kT[64:128, csl], in_=kdst)
                    return f

                def u_v_t(sc, get):
                    def f():
                        kvraw = get()
                        vst = sbA.tile([128, 4, 64], BF16, name="vst",
                                       tag="vst", bufs=2)
                        nc.sync.dma_start_transpose(
                            out=vst[:, :, :], in_=kvraw[64:128, :])
                        nc.vector.tensor_copy(vsm[:, 4 * sc:4 * sc + 4, 0:64],
                                              vst[:, :, :])
                    return f

                def u_q(sc, pp):
                    csl = slice(512 * sc, 512 * sc + 512)

                    st = {}

                    wqs = wq_sbs[pp]

                    def f1():
                        ps = psD.tile([128, 512], F32, name="ps_q",
                                      tag="ps_d", bufs=2)
                        for dc in range(4):
                            nc.tensor.matmul(
                                ps, wqs[:, dc // 2,
                                        128 * (dc % 2):128 * (dc % 2) + 128],
                                ht_sb[:, dc, csl],
                                start=(dc == 0), stop=False)
                        st["ps"] = ps

                    def f2():
                        ps = st["ps"]
                        for dc in range(4, 8):
                            nc.tensor.matmul(
                                ps, wqs[:, dc // 2,
                                        128 * (dc % 2):128 * (dc % 2) + 128],
                                ht_sb[:, dc, csl],
                                start=False, stop=(dc == 7))
                        qraw = sbA.tile([128, 512], BF16, name="qraw",
                                        tag="qraw", bufs=3)
                        nc.vector.tensor_copy(qraw, ps)
                        return qraw
                    return f1, f2

                def u_q_rope(sc, pp, get):
                    csl = slice(512 * sc, 512 * sc + 512)

                    def f():
                        qraw = get()
                        ps_rot = psD.tile([128, 512], F32, name="ps_rot",
                                          tag="ps_d", bufs=2)
                        nc.tensor.matmul(ps_rot, psig_sb, qraw,
                                         start=True, stop=True)
                        dst = qTs[pp][:, csl]
                        nc.gpsimd.tensor_mul(dst, qraw, cosf[:, csl])
                        rtmp = sbA.tile([128, 512], BF16, name="rtmp",
                                        tag="rtmp", bufs=3)
                        nc.vector.tensor_mul(rtmp, ps_rot, sinf[:, csl])
                        nc.gpsimd.tensor_add(dst, dst, rtmp)
                    return f

                def strip_units(sc):
                    box = {}

                    def mk(key, fn):
                        def g():
                            box[key] = fn()
                        return g

                    def rd(key):
                        return lambda: box[key]

                    kv1, kv2 = u_kv(sc)
                    q01, q02 = u_q(sc, 0)
                    q11, q12 = u_q(sc, 1)

                    return [
                        kv1,
                        mk("kv", kv2),
                        q01,
                        mk("q0", q02),
                        u_kv_rope(sc, rd("kv")),
                        u_q_rope(sc, 0, rd("q0")),
                        u_v_t(sc, rd("kv")),
                        q11,
                        mk("q1", q12),
                        u_q_rope(sc, 1, rd("q1")),
                    ]

                # ---------------- attention units ----------------
                # per (qb, sp): heads hA=2sp, hB=2sp+1 interleaved
                def bc_units(qb, only_sp=None):
                    qsl = slice(512 * qb, 512 * qb + 512)
                    units = []
                    sps = (1, 0) if qb == NQB - 1 else (0, 1)
                    if only_sp is not None:
                        sps = (only_sp,)
                    for sp in sps:
                        hs = [2 * sp, 2 * sp + 1]
                        ctxps = {}
                        nki = (4 * qb + 4) if causal else NT
                        nfull = (4 * qb) if causal else NT

                        def mk_ctx_alloc(h):
                            def g():
                                ctxps[h] = psC.tile([128, 512], F32,
                                                    name="ps_ctx",
                                                    tag="ps_ctx", bufs=2)
                            return g

                        # scores+exp+ctx closures
                        def mk_pair_s(h, kp, bx):
                            off = 64 * (h % 2)
                            pp = h // 2

                            def g():
                                ps = psS.tile([128, 1024], F32, name="ps_s",
                                              tag="ps_s", bufs=2)
                                for jj in range(2):
                                    ki = kp + jj
                                    nc.tensor.matmul(
                                        ps[:, 512 * jj:512 * jj + 512],
                                        kT[off:off + 64,
                                           128 * ki:128 * ki + 128],
                                        qTs[pp][off:off + 64, qsl],
                                        start=True, stop=True)
                                bx["ps"] = ps
                            return g

                        def mk_pair_e(h, kp, bx, _nki=nki):
                            def g():
                                et = etp.tile([128, 1024], BF16, name="et",
                                              tag="et", bufs=6)
                                nc.scalar.activation(et, bx["ps"], AF.Exp,
                                                     scale=SCALE)
                                for jj in range(2):
                                    ki = kp + jj
                                    nc.tensor.matmul(
                                        ctxps[h][0:65, :],
                                        vsm[:, ki, 0:65],
                                        et[:, 512 * jj:512 * jj + 512],
                                        start=(ki == 0),
                                        stop=(ki == _nki - 1))
                            return g

                        # diag group: js = (0,1) or (2,3); slot offsets
                        def mk_diag_s(h, js, bx):
                            off = 64 * (h % 2)
                            pp = h // 2

                            def g():
                                spans = [512 - 128 * j for j in js]
                                tot = sum(spans)
                                ps = psS.tile([128, 1024], F32, name="ps_g",
                                              tag="ps_s", bufs=2)
                                o = 0
                                for j, span in zip(js, spans):
                                    ki = 4 * qb + j
                                    # diagonal 128-col block: scores then the
                                    # folded causal mask (+= I^T @ -1e9 tri)
                                    nc.tensor.matmul(
                                        ps[:, o:o + 128],
                                        kT[off:off + 64,
                                           128 * ki:128 * ki + 128],
                                        qTs[pp][off:off + 64,
                                                512 * qb + 128 * j:
                                                512 * qb + 128 * j + 128],
                                        start=True, stop=False)
                                    nc.tensor.matmul(
                                        ps[:, o:o + 128], id_sb, m01_sb,
                                        start=False, stop=True)
                                    if span > 128:
                                        # fully-causal remainder of the span
                                        nc.tensor.matmul(
                                            ps[:, o + 128:o + span],
                                            kT[off:off + 64,
                                               128 * ki:128 * ki + 128],
                                            qTs[pp][off:off + 64,
                                                    512 * qb + 128 * (j + 1):
                                                    512 * qb + 512],
                                            start=True, stop=True)
                                    o += span
                                bx["ps"] = ps
                                bx["tot"] = tot
                            return g

                        def mk_diag_e(h, js, bx, _nki=nki):
                            def g():
                                spans = [512 - 128 * j for j in js]
                                et = etp.tile([128, 1024], BF16, name="etd",
                                              tag="et", bufs=6)
                                nc.scalar.activation(et[:, 0:bx["tot"]],
                                                     bx["ps"][:, 0:bx["tot"]],
                                                     AF.Exp, scale=SCALE)
                                o = 0
                                for j, span in zip(js, spans):
                                    ki = 4 * qb + j
                                    nc.tensor.matmul(
                                        ctxps[h][0:65, 128 * j:512],
                                        vsm[:, ki, 0:65],
                                        et[:, o:o + span],
                                        start=(ki == 0),
                                        stop=(ki == _nki - 1))
                                    o += span
                            return g

                        def mk_norm(h):
                            c = h % 2
                            up = h // 2   # 0: partitions 0:64, 1: 64:128

                            def g():
                                pc = ctxps[h]
                                ctxu = sbC.tile([65, 512], BF16, name="ctxu",
                                                tag="ctxu", bufs=4)
                                nc.vector.tensor_copy(ctxu, pc[0:65, :])
                                nc.vector.reciprocal(ctxu[64:65, :],
                                                     ctxu[64:65, :])
                                ps_b = psD.tile([128, 512], F32, name="ps_b",
                                                tag="ps_d", bufs=2)[0:64, :]
                                nc.tensor.matmul(
                                    ps_b, ones_sb[64:65, :],
                                    ctxu[64:65, :],
                                    start=True, stop=True)
                                if up == 0:
                                    nc.vector.tensor_mul(
                                        ctxTs[c][qb][0:64, :],
                                        ctxu[0:64, :], ps_b)
                                else:
                                    ctmp = sbC.tile([64, 512], BF16,
                                                    name="ctmp", tag="ctmp",
                                                    bufs=2)
                                    nc.vector.tensor_mul(ctmp, ctxu[0:64, :],
                                                         ps_b)
                                    nc.sync.dma_start(
                                        out=ctxTs[c][qb][64:128, :], in_=ctmp)
                            return g

                        # build interleaved 2-head stream for this sp
                        hA, hB = hs
                        units.append(mk_ctx_alloc(hA))
                        units.append(mk_ctx_alloc(hB))
                        boxes = {h: [] for h in hs}
                        sA, eA, sB, eB = [], [], [], []
                        for kp in range(0, nfull, 2):
                            bxA, bxB = {}, {}
                            sA.append(mk_pair_s(hA, kp, bxA))
                            eA.append(mk_pair_e(hA, kp, bxA))
                            sB.append(mk_pair_s(hB, kp, bxB))
                            eB.append(mk_pair_e(hB, kp, bxB))
                        if causal:
                            for js in ((0, 1), (2, 3)):
                                bxA, bxB = {}, {}
                                sA.append(mk_diag_s(hA, js, bxA))
                                eA.append(mk_diag_e(hA, js, bxA))
                                sB.append(mk_diag_s(hB, js, bxB))
                                eB.append(mk_diag_e(hB, js, bxB))
                        # pipeline: sA0 sB0 | eA0 sA1 eB0 sB1 | eA1 sA2 ...
                        n = len(sA)
                        if n:
                            units.append(sA[0])
                            units.append(sB[0])
                            for i in range(n):
                                if i + 1 < n:
                                    units.append(eA[i])
                                    units.append(sA[i + 1])
                                    units.append(eB[i])
                                    units.append(sB[i + 1])
                                else:
                                    units.append(eA[i])
                                    units.append(eB[i])
                        units.append(mk_norm(hA))
                        units.append(mk_norm(hB))
                    return units

                # ---------------- phase D units ----------------
                def d_units(qb, use_act=False):
                    units = []
                    for qt in range(4):
                        def mk(qt=qt):
                            rows = slice(512 * qb + 128 * qt,
                                         512 * qb + 128 * qt + 128)
                            col = 128 * qt

                            def g():
                                ost = sbC.tile([128, 1024], BF16, name="ost",
                                               tag="ost", bufs=3)
                                split = use_act
                                for nb in range(2):
                                    ps_o = psD.tile([128, 512], F32,
                                                    name="ps_o", tag="ps_d",
                                                    bufs=2)
                                    for c in range(2):
                                        nc.tensor.matmul(
                                            ps_o,
                                            ctxTs[c][qb][:, col:col + 128],
                                            wo_sb[:, c,
                                                  512 * nb:512 * nb + 512],
                                            start=(c == 0), stop=(c == 1))
                                    dst = ost[:, 512 * nb:512 * nb + 512]
                                    if use_act and nb == 0:
                                        nc.scalar.copy(dst, ps_o)
                                    else:
                                        nc.vector.tensor_copy(dst, ps_o)
                                    if split:
                                        nc.sync.dma_start(
                                            out=outp[rows,
                                                     512 * nb:512 * nb + 512],
                                            in_=dst)
                                if not split:
                                    nc.sync.dma_start(out=outp[rows, :],
                                                      in_=ost)
                            return g
                        units.append(mk())
                    return units

                # ---------------- global emission ----------------
                def lab_units(units, pfx):
                    out = []
                    for i, u in enumerate(units):
                        def w(u=u, l=f"{pfx}.{i}"):
                            nc._set_label(l)
                            u()
                        out.append(w)
                    return out

                for u in lab_units(strip_units(0), "strip0"):
                    u()
                for qb in range(NQB):
                    bcu = lab_units(bc_units(qb), f"bc{qb}")
                    fill = []
                    if qb == 0:
                        fill += lab_units(strip_units(1), "strip1")
                    if qb == 1:
                        fill += lab_units(strip_units(2), "strip2")
                    if qb == 2:
                        fill += lab_units(strip_units(3), "strip3")
                    if qb == 3:
                        fill += lab_units(d_units(0), "d0")
                        fill += lab_units(d_units(1), "d1")
                        fill += lab_units(d_units(2), "d2")
                    for u in _interleave(bcu, fill):
                        u()
                for u in lab_units(d_units(3, use_act=True), "d3"):
                    u()

    nc.compile()
    return nc


_NC_CACHE = {}


def _get_nc(causal: bool):
    if causal not in _NC_CACHE:
        _NC_CACHE[causal] = _build_nc(causal)
    return _NC_CACHE[causal]


def _host_consts():
    p = np.zeros((128, 128), np.float32)
    idx = np.arange(0, 128, 2)
    p[idx, idx + 1] = -1.0
    p[idx + 1, idx] = 1.0
    psigT = np.ascontiguousarray(p.T)
    ident = np.eye(128, dtype=np.float32)
    m01n = np.where(np.arange(128)[None, :] >= np.arange(128)[:, None],
                    0.0, NEG).astype(np.float32)
    return psigT, ident, m01n


def _numpy_reference(hidden_states, cos, sin, attention_mask, Wq, Wk, Wv, Wo):
    """Generic-mask fallback, pure numpy port of the reference."""
    GROUPS = H // KVH

    def rope(x, c, s):
        c = c[:, None, :, :]
        s = s[:, None, :, :]
        x1, x2 = x[..., ::2], x[..., 1::2]
        xr = np.stack([x1 * c - x2 * s, x1 * s + x2 * c], axis=-1)
        return xr.reshape(x.shape)

    b, sq, d = hidden_states.shape
    q = (hidden_states @ Wq).reshape(b, sq, H, HD).transpose(0, 2, 1, 3)
    k = (hidden_states @ Wk).reshape(b, sq, KVH, HD).transpose(0, 2, 1, 3)
    v = (hidden_states @ Wv).reshape(b, sq, KVH, HD).transpose(0, 2, 1, 3)
    q = rope(q, cos, sin)
    k = rope(k, cos, sin)
    k = np.repeat(k, GROUPS, axis=1)
    v = np.repeat(v, GROUPS, axis=1)
    out = np.zeros((b, sq, d), np.float32)
    for bi in range(b):
        for hi in range(H):
            sc = (q[bi, hi] @ k[bi, hi].T) * SCALE + attention_mask[0, 0]
            sc = sc - sc.max(axis=-1, keepdims=True)
            e = np.exp(sc)
            pr = e / e.sum(axis=-1, keepdims=True)
            ctx = pr @ v[bi, hi]
            out[bi] += ctx @ Wo[hi * HD:(hi + 1) * HD]
    return out


def kernel(**inputs) -> np.ndarray:
    hs = np.asarray(inputs["hidden_states"], np.float32)
    cos = np.asarray(inputs["cos"], np.float32)
    sin = np.asarray(inputs["sin"], np.float32)
    mask = np.asarray(inputs["attention_mask"], np.float32)
    Wq = np.asarray(inputs["Wq"], np.float32)
    Wk = np.asarray(inputs["Wk"], np.float32)
    Wv = np.asarray(inputs["Wv"], np.float32)
    Wo = np.asarray(inputs["Wo"], np.float32)

    m = mask.reshape(S, S)
    tril = np.tril(np.ones((S, S), dtype=bool))
    causal_ref = np.where(tril, np.float32(0.0), np.float32(NEG))
    if np.array_equal(m, causal_ref):
        causal = True
    elif not m.any():
        causal = False
    else:
        return _numpy_reference(hs, cos, sin, mask, Wq, Wk, Wv, Wo)

    nc = _get_nc(causal)
    psigT, ident, m01 = _host_consts()
    chan_half = (np.arange(64) // 2)
    bf = ml_dtypes.bfloat16

    in_maps = []
    for core in range(8):
        b, t = core // TP, core % TP
        hT = np.ascontiguousarray(hs[b].T).astype(bf)
        hT3 = np.ascontiguousarray(
            hT.reshape(8, 128, S).transpose(1, 0, 2))
        cs_v = np.ascontiguousarray(cos[b].T[chan_half, :]).astype(bf)
        sn_v = np.ascontiguousarray(sin[b].T[chan_half, :]).astype(bf)
        wq_s = Wq[:, t * 256:(t + 1) * 256].astype(bf)
        wq4 = wq_s.reshape(8, 128, 2, 128).transpose(1, 0, 2, 3)
        wqA_h = np.ascontiguousarray(wq4[:, :, 0, :].reshape(128, 4, 256))
        wqB_h = np.ascontiguousarray(wq4[:, :, 1, :].reshape(128, 4, 256))
        wkv_s = np.concatenate([Wk[:, t * 64:(t + 1) * 64],
                                Wv[:, t * 64:(t + 1) * 64]], axis=1).astype(bf)
        wkv3 = np.ascontiguousarray(
            wkv_s.reshape(8, 128, 128).transpose(1, 0, 2)
            .reshape(128, 4, 256))
        wo_s = Wo[t * 256:(t + 1) * 256]
        # ctxT channel order per chunk: c0 = [h0|h2], c1 = [h1|h3]
        wo_p = np.concatenate([wo_s[0:64], wo_s[128:192],
                               wo_s[64:128], wo_s[192:256]], axis=0).astype(bf)
        wo3 = np.ascontiguousarray(wo_p.reshape(2, 128, D).transpose(1, 0, 2))
        in_maps.append({
            "hT3": hT3, "cs": cs_v, "sn": sn_v,
            "wqA": wqA_h, "wqB": wqB_h, "wkv3": wkv3, "wo3": wo3,
            "psigT": psigT.astype(bf), "ident": ident.astype(bf),
            "m01": m01.astype(bf),
            "onesc": np.ones((128, 64), bf),
        })

    res = run_bass_kernel_spmd(nc, in_maps, core_ids=list(range(8)))
    out = np.zeros((B, S, D), np.float32)
    for core in range(8):
        out[core // TP] += np.asarray(res.results[core]["out"],
                                      dtype=np.float32)
    return out


# revision 7
# speedup vs baseline: 1.0148x; 1.0000x over previous
"""Self-contained Trainium2 Bass kernel for GQA MultiHeadAttention with RoPE.

Problem: B=2, S=2048, D=1024, H=16 Q heads, KVH=4 KV heads, head_dim=64,
causal additive mask, f32 in/out.

Sharding: TP=4 over heads (4 Q heads + 1 KV head per shard) x DP=2 over
batch = 8 NeuronCores. Wo sharded on input dim; host sums the 4 partial
outputs per batch element.

Design notes (v2):
- All-bf16 datapath (weights, hidden, q/k/v, exp probs, ctx, out partials);
  psum accumulation stays f32. Error budget 2e-2 rel leaves ~7x headroom.
- Phase A (projections+rope) is emitted strip-wise (512 seq cols) and
  interleaved with attention blocks so PE never idles on exp latency.
- Softmax normalization is fused on-psum: DVE reciprocal of the rowsum row
  (row 64 of the ctx psum, produced by an ones-column in vsm), PE ones-matmul
  broadcast of the recip row, DVE multiply straight out of psum into ctxT.
  No DRAM scratch, no reshape bounces.
- Diagonal (causal-edge) tiles are batched into 2 grouped psum slots per
  (head, qb) so exps are [128, 896]+[128, 384] instead of 4 small ones.
- DMAs are coalesced (one per hT strip, one out-store per 128-row tile) and
  issued on the engine that produced their source.
"""

import os
import sys

for _p in ("/opt/trn_rl_repo", "/root/.axon_site/_ro/trn_rl_repo"):
    if os.path.isdir(_p) and _p not in sys.path:
        sys.path.insert(0, _p)

import numpy as np
import ml_dtypes

import concourse.bacc as bacc
import concourse.bass as bass
import concourse.tile as tile
from concourse import mybir
from concourse.bass_utils import run_bass_kernel_spmd

F32 = mybir.dt.float32
F32R = mybir.dt.float32r
BF16 = mybir.dt.bfloat16
AF = mybir.ActivationFunctionType

H, KVH, HD = 16, 4, 64
B, S, D = 2, 2048, 1024
TP = 4
SCALE = HD ** -0.5
NEG = -1e9
NT = S // 128               # 16 kv tiles
NQB = S // 512              # 4 q blocks


def _interleave(a, b):
    """Merge two unit lists proportionally (a is the primary stream)."""
    if not b:
        return list(a)
    if not a:
        return list(b)
    out = []
    na, nb = len(a), len(b)
    ia = ib = 0
    while ia < na or ib < nb:
        # emit whichever stream is behind proportionally
        if ib >= nb or (ia < na and ia * nb <= ib * na):
            out.append(a[ia]); ia += 1
        else:
            out.append(b[ib]); ib += 1
    return out


MM_LABELS = []


def _build_nc(causal: bool):
    nc = bacc.Bacc()
    MM_LABELS.clear()
    _orig_mm = nc.tensor.matmul
    _cur = {"l": "?"}

    def _mm(*a, **kw):
        MM_LABELS.append(_cur["l"])
        return _orig_mm(*a, **kw)
    nc.tensor.matmul = _mm

    def _lab(s):
        _cur["l"] = s
    nc._set_label = _lab

    hT3 = nc.declare_dram_parameter("hT3", [128, 8, S], BF16, isOutput=False)
    cs = nc.declare_dram_parameter("cs", [64, S], BF16, isOutput=False)
    sn = nc.declare_dram_parameter("sn", [64, S], BF16, isOutput=False)
    wqA = nc.declare_dram_parameter("wqA", [128, 4, 256], BF16, isOutput=False)
    wqB = nc.declare_dram_parameter("wqB", [128, 4, 256], BF16, isOutput=False)
    wkv3 = nc.declare_dram_parameter("wkv3", [128, 4, 256], BF16, isOutput=False)
    wo3 = nc.declare_dram_parameter("wo3", [128, 2, D], BF16, isOutput=False)
    psigT = nc.declare_dram_parameter("psigT", [128, 128], BF16, isOutput=False)
    ident = nc.declare_dram_parameter("ident", [128, 128], BF16, isOutput=False)
    m01 = nc.declare_dram_parameter("m01", [128, 128], BF16, isOutput=False)
    onesc = nc.declare_dram_parameter("onesc", [128, 64], BF16, isOutput=False)
    outp = nc.declare_dram_parameter("out", [S, D], BF16, isOutput=True)

    with tile.TileContext(nc) as tc, nc.allow_low_precision(
            reason="2e-2 rel tolerance; bf16 throughout"):
        with tc.tile_pool(name="hold", bufs=1) as hp:
            wkv_sb = hp.tile([128, 4, 256], BF16, name="wkv_sb", tag="wkv_sb")
            psig_sb = hp.tile([128, 128], BF16, name="psig_sb", tag="psig_sb")
            cosf = hp.tile([128, S], BF16, name="cosf", tag="cosf")
            sinf = hp.tile([128, S], BF16, name="sinf", tag="sinf")
            wq_sbs = [hp.tile([128, 4, 256], BF16, name=f"wq_sb{i}",
                              tag=f"wq_sb{i}") for i in range(2)]
            id_sb = hp.tile([128, 128], BF16, name="id_sb", tag="id_sb")
            m01_sb = hp.tile([128, 128], BF16, name="m01_sb", tag="m01_sb")
            ones_sb = hp.tile([128, 64], BF16, name="ones_sb", tag="ones_sb")
            wo_sb = hp.tile([128, 2, D], BF16, name="wo_sb", tag="wo_sb")
            ht_sb = hp.tile([128, 8, S], BF16, name="ht_sb", tag="ht_sb")
            qTs = [hp.tile([128, S], BF16, name=f"qT{p}", tag=f"qT{p}")
                   for p in range(2)]
            kT = hp.tile([128, S], BF16, name="kTt", tag="kTt")
            vsm = hp.tile([128, NT, 65], BF16, name="vsm", tag="vsm")
            ctxTs = [[hp.tile([128, 512], BF16, name=f"ctxT{c}_{q}",
                              tag=f"ctxT{c}_{q}") for q in range(NQB)]
                     for c in range(2)]

            # ---- prologue DMAs (SP queue), ordered by first use ----
            nc.sync.dma_start(out=wkv_sb, in_=wkv3[:, :, :])
            nc.sync.dma_start(out=ht_sb[:, 0:2, 0:512], in_=hT3[:, 0:2, 0:512])
            nc.sync.dma_start(out=ht_sb[:, 2:4, 0:512], in_=hT3[:, 2:4, 0:512])
            nc.sync.dma_start(out=ht_sb[:, 4:6, 0:512], in_=hT3[:, 4:6, 0:512])
            nc.sync.dma_start(out=ht_sb[:, 6:8, 0:512], in_=hT3[:, 6:8, 0:512])
            nc.sync.dma_start(out=psig_sb, in_=psigT[:, :])
            nc.sync.dma_start(out=cosf[0:64, :], in_=cs[:, :])
            nc.sync.dma_start(out=cosf[64:128, :], in_=cs[:, :])
            nc.sync.dma_start(out=sinf[0:64, :], in_=sn[:, :])
            nc.sync.dma_start(out=sinf[64:128, :], in_=sn[:, :])
            nc.sync.dma_start(out=wq_sbs[0], in_=wqA[:, :, :])
            nc.sync.dma_start(out=wq_sbs[1], in_=wqB[:, :, :])
            nc.sync.dma_start(out=ht_sb[:, :, 512:1024], in_=hT3[:, :, 512:1024])
            nc.sync.dma_start(out=id_sb, in_=ident[:, :])
            nc.sync.dma_start(out=m01_sb, in_=m01[:, :])
            nc.sync.dma_start(out=ones_sb, in_=onesc[:, :])
            nc.sync.dma_start(out=wo_sb, in_=wo3[:, :, :])
            nc.sync.dma_start(out=ht_sb[:, :, 1024:1536], in_=hT3[:, :, 1024:1536])
            nc.sync.dma_start(out=ht_sb[:, :, 1536:2048], in_=hT3[:, :, 1536:2048])
            # ones column of vsm for the softmax denominator row
            nc.gpsimd.memset(vsm[:, :, 64:65], 1.0)
            # warm the ACT exp table while ACT is idle (the implicit load
            # would otherwise land on the first attention exp)
            actw = hp.tile([1, 16], F32, name="actw", tag="actw")
            nc.gpsimd.memset(actw[:, :], 1.0)
            nc.scalar.activation(actw, actw, AF.Exp, scale=1.0)


            with tc.tile_pool(name="psS", bufs=1, space="PSUM") as psS, \
                 tc.tile_pool(name="psD", bufs=1, space="PSUM") as psD, \
                 tc.tile_pool(name="psC", bufs=1, space="PSUM") as psC, \
                 tc.tile_pool(name="etp", bufs=1) as etp, \
                 tc.tile_pool(name="sbA", bufs=1) as sbA, \
                 tc.tile_pool(name="sbC", bufs=1) as sbC:

                # ---------------- phase A units (per strip) ----------------
                def u_kv(sc):
                    csl = slice(512 * sc, 512 * sc + 512)

                    st = {}

                    def f1():
                        ps = psD.tile([128, 512], F32, name="ps_kv",
                                      tag="ps_d", bufs=2)
                        for dc in range(4):
                            nc.tensor.matmul(
                                ps, wkv_sb[:, dc // 2,
                                           128 * (dc % 2):128 * (dc % 2) + 128],
                                ht_sb[:, dc, csl],
                                start=(dc == 0), stop=False)
                        st["ps"] = ps

                    def f2():
                        ps = st["ps"]
                        for dc in range(4, 8):
                            nc.tensor.matmul(
                                ps, wkv_sb[:, dc // 2,
                                           128 * (dc % 2):128 * (dc % 2) + 128],
                                ht_sb[:, dc, csl],
                                start=False, stop=(dc == 7))
                        kvraw = sbA.tile([128, 512], BF16, name="kvraw",
                                         tag="kvraw", bufs=3)
                        nc.vector.tensor_copy(kvraw, ps)
                        return kvraw
                    return f1, f2

                def u_kv_rope(sc, get):
                    csl = slice(512 * sc, 512 * sc + 512)

                    def f():
                        kvraw = get()
                        ps_kr = psD.tile([128, 512], F32, name="ps_kr",
                                         tag="ps_d", bufs=2)[0:64, :]
                        nc.tensor.matmul(ps_kr, psig_sb[0:64, 0:64],
                                         kvraw[0:64, :], start=True, stop=True)
                        kdst = kT[0:64, csl]
                        nc.gpsimd.tensor_mul(kdst, kvraw[0:64, :],
                                             cosf[0:64, csl])
                        ktmp = sbA.tile([64, 512], BF16, name="ktmp",
                                        tag="ktmp", bufs=3)
                        nc.vector.tensor_mul(ktmp, ps_kr, sinf[0:64, csl])
                        nc.gpsimd.tensor_add(kdst, kdst, ktmp)
                        # duplicate roped K to partitions 64:128 (odd heads)
                        nc.sync.dma_start(out=kT[64:128, csl], in_=kdst)
                    return f

                def u_v_t(sc, get):
                    def f():
                        kvraw = get()
                        vst = sbA.tile([128, 4, 64], BF16, name="vst",
                                       tag="vst", bufs=3)
                        nc.sync.dma_start_transpose(
                            out=vst[:, :, :], in_=kvraw[64:128, :])
                        nc.vector.tensor_copy(vsm[:, 4 * sc:4 * sc + 4, 0:64],
                                              vst[:, :, :])
                    return f

                def u_q(sc, pp):
                    csl = slice(512 * sc, 512 * sc + 512)

                    st = {}

                    wqs = wq_sbs[pp]

                    def f1():
                        ps = psD.tile([128, 512], F32, name="ps_q",
                                      tag="ps_d", bufs=2)
                        for dc in range(4):
                            nc.tensor.matmul(
                                ps, wqs[:, dc // 2,
                                        128 * (dc % 2):128 * (dc % 2) + 128],
                                ht_sb[:, dc, csl],
                                start=(dc == 0), stop=False)
                        st["ps"] = ps

                    def f2():
                        ps = st["ps"]
                        for dc in range(4, 8):
                            nc.tensor.matmul(
                                ps, wqs[:, dc // 2,
                                        128 * (dc % 2):128 * (dc % 2) + 128],
                                ht_sb[:, dc, csl],
                                start=False, stop=(dc == 7))
                        qraw = sbA.tile([128, 512], BF16, name="qraw",
                                        tag="qraw", bufs=3)
                        nc.vector.tensor_copy(qraw, ps)
                        return qraw
                    return f1, f2

                def u_q_rope(sc, pp, get):
                    csl = slice(512 * sc, 512 * sc + 512)

                    def f():
                        qraw = get()
                        ps_rot = psD.tile([128, 512], F32, name="ps_rot",
                                          tag="ps_d", bufs=2)
                        nc.tensor.matmul(ps_rot, psig_sb, qraw,
                                         start=True, stop=True)
                        dst = qTs[pp][:, csl]
                        nc.gpsimd.tensor_mul(dst, qraw, cosf[:, csl])
                        rtmp = sbA.tile([128, 512], BF16, name="rtmp",
                                        tag="rtmp", bufs=3)
                        nc.vector.tensor_mul(rtmp, ps_rot, sinf[:, csl])
                        nc.gpsimd.tensor_add(dst, dst, rtmp)
                    return f

                def strip_units(sc):
                    box = {}

                    def mk(key, fn):
                        def g():
                            box[key] = fn()
                        return g

                    def rd(key):
                        return lambda: box[key]

                    kv1, kv2 = u_kv(sc)
                    q01, q02 = u_q(sc, 0)
                    q11, q12 = u_q(sc, 1)

                    return [
                        kv1,
                        mk("kv", kv2),
                        q01,
                        mk("q0", q02),
                        u_kv_rope(sc, rd("kv")),
                        u_q_rope(sc, 0, rd("q0")),
                        u_v_t(sc, rd("kv")),
                        q11,
                        mk("q1", q12),
                        u_q_rope(sc, 1, rd("q1")),
                    ]

                # ---------------- attention units ----------------
                # per (qb, sp): heads hA=2sp, hB=2sp+1 interleaved
                def bc_units(qb, only_sp=None):
                    qsl = slice(512 * qb, 512 * qb + 512)
                    units = []
                    sps = (1, 0) if qb == NQB - 1 else (0, 1)
                    if only_sp is not None:
                        sps = (only_sp,)
                    for sp in sps:
                        hs = [2 * sp, 2 * sp + 1]
                        ctxps = {}
                        nki = (4 * qb + 4) if causal else NT
                        nfull = (4 * qb) if causal else NT

                        def mk_ctx_alloc(h):
                            def g():
                                ctxps[h] = psC.tile([128, 512], F32,
                                                    name="ps_ctx",
                                                    tag="ps_ctx", bufs=2)
                            return g

                        # scores+exp+ctx closures
                        def mk_pair_s(h, kp, bx):
                            off = 64 * (h % 2)
                            pp = h // 2

                            def g():
                                ps = psS.tile([128, 1024], F32, name="ps_s",
                                              tag="ps_s", bufs=2)
                                for jj in range(2):
                                    ki = kp + jj
                                    nc.tensor.matmul(
                                        ps[:, 512 * jj:512 * jj + 512],
                                        kT[off:off + 64,
                                           128 * ki:128 * ki + 128],
                                        qTs[pp][off:off + 64, qsl],
                                        start=True, stop=True)
                                bx["ps"] = ps
                            return g

                        def mk_pair_e(h, kp, bx, _nki=nki):
                            def g():
                                et = etp.tile([128, 1024], BF16, name="et",
                                              tag="et", bufs=6)
                                nc.scalar.activation(et, bx["ps"], AF.Exp,
                                                     scale=SCALE)
                                for jj in range(2):
                                    ki = kp + jj
                                    nc.tensor.matmul(
                                        ctxps[h][0:65, :],
                                        vsm[:, ki, 0:65],
                                        et[:, 512 * jj:512 * jj + 512],
                                        start=(ki == 0),
                                        stop=(ki == _nki - 1))
                            return g

                        # diag group: js = (0,1) or (2,3); slot offsets
                        def mk_diag_s(h, js, bx):
                            off = 64 * (h % 2)
                            pp = h // 2

                            def g():
                                spans = [512 - 128 * j for j in js]
                                tot = sum(spans)
                                ps = psS.tile([128, 1024], F32, name="ps_g",
                                              tag="ps_s", bufs=2)
                                o = 0
                                for j, span in zip(js, spans):
                                    ki = 4 * qb + j
                                    # diagonal 128-col block: scores then the
                                    # folded causal mask (+= I^T @ -1e9 tri)
                                    nc.tensor.matmul(
                                        ps[:, o:o + 128],
                                        kT[off:off + 64,
                                           128 * ki:128 * ki + 128],
                                        qTs[pp][off:off + 64,
                                                512 * qb + 128 * j:
                                                512 * qb + 128 * j + 128],
                                        start=True, stop=False)
                                    nc.tensor.matmul(
                                        ps[:, o:o + 128], id_sb, m01_sb,
                                        start=False, stop=True)
                                    if span > 128:
                                        # fully-causal remainder of the span
                                        nc.tensor.matmul(
                                            ps[:, o + 128:o + span],
                                            kT[off:off + 64,
                                               128 * ki:128 * ki + 128],
                                            qTs[pp][off:off + 64,
                                                    512 * qb + 128 * (j + 1):
                                                    512 * qb + 512],
                                            start=True, stop=True)
                                    o += span
                                bx["ps"] = ps
                                bx["tot"] = tot
                            return g

                        def mk_diag_e(h, js, bx, _nki=nki):
                            def g():
                                spans = [512 - 128 * j for j in js]
                                et = etp.tile([128, 1024], BF16, name="etd",
                                              tag="et", bufs=6)
                                nc.scalar.activation(et[:, 0:bx["tot"]],
                                                     bx["ps"][:, 0:bx["tot"]],
                                                     AF.Exp, scale=SCALE)
                                o = 0
                                for j, span in zip(js, spans):
                                    ki = 4 * qb + j
                                    nc.tensor.matmul(
                                        ctxps[h][0:65, 128 * j:512],
                                        vsm[:, ki, 0:65],
                                        et[:, o:o + span],
                                        start=(ki == 0),
                                        stop=(ki == _nki - 1))
                                    o += span
                            return g

                        def mk_norm(h):
                            c = h % 2
                            up = h // 2   # 0: partitions 0:64, 1: 64:128

                            def g():
                                pc = ctxps[h]
                                ctxu = sbC.tile([65, 512], BF16, name="ctxu",
                                                tag="ctxu", bufs=4)
                                nc.vector.tensor_copy(ctxu, pc[0:65, :])
                                nc.vector.reciprocal(ctxu[64:65, :],
                                                     ctxu[64:65, :])
                                ps_b = psD.tile([128, 512], F32, name="ps_b",
                                                tag="ps_d", bufs=2)[0:64, :]
                                nc.tensor.matmul(
                                    ps_b, ones_sb[64:65, :],
                                    ctxu[64:65, :],
                                    start=True, stop=True)
                                if up == 0:
                                    nc.vector.tensor_mul(
                                        ctxTs[c][qb][0:64, :],
                                        ctxu[0:64, :], ps_b)
                                else:
                                    ctmp = sbC.tile([64, 512], BF16,
                                                    name="ctmp", tag="ctmp",
                                                    bufs=3)
                                    nc.vector.tensor_mul(ctmp, ctxu[0:64, :],
                                                         ps_b)
                                    nc.sync.dma_start(
                                        out=ctxTs[c][qb][64:128, :], in_=ctmp)
                            return g

                        # build interleaved 2-head stream for this sp
                        hA, hB = hs
                        units.append(mk_ctx_alloc(hA))
                        units.append(mk_ctx_alloc(hB))
                        boxes = {h: [] for h in hs}
                        sA, eA, sB, eB = [], [], [], []
                        for kp in range(0, nfull, 2):
                            bxA, bxB = {}, {}
                            sA.append(mk_pair_s(hA, kp, bxA))
                            eA.append(mk_pair_e(hA, kp, bxA))
                            sB.append(mk_pair_s(hB, kp, bxB))
                            eB.append(mk_pair_e(hB, kp, bxB))
                        if causal:
                            for js in ((0, 1), (2, 3)):
                                bxA, bxB = {}, {}
                                sA.append(mk_diag_s(hA, js, bxA))
                                eA.append(mk_diag_e(hA, js, bxA))
                                sB.append(mk_diag_s(hB, js, bxB))
                                eB.append(mk_diag_e(hB, js, bxB))
                        # pipeline: sA0 sB0 | eA0 sA1 eB0 sB1 | eA1 sA2 ...
                        n = len(sA)
                        if n:
                            units.append(sA[0])
                            units.append(sB[0])
                            for i in range(n):
                                if i + 1 < n:
                                    units.append(eA[i])
                                    units.append(sA[i + 1])
                                    units.append(eB[i])
                                    units.append(sB[i + 1])
                                else:
                                    units.append(eA[i])
                                    units.append(eB[i])
                        units.append(mk_norm(hA))
                        units.append(mk_norm(hB))
                    return units

                # ---------------- phase D units ----------------
                def d_units(qb, use_act=False):
                    units = []
                    for qt in range(4):
                        def mk(qt=qt):
                            rows = slice(512 * qb + 128 * qt,
                                         512 * qb + 128 * qt + 128)
                            col = 128 * qt

                            def g():
                                ost = sbC.tile([128, 1024], BF16, name="ost",
                                               tag="ost", bufs=4)
                                split = use_act
                                for nb in range(2):
                                    ps_o = psD.tile([128, 512], F32,
                                                    name="ps_o", tag="ps_d",
                                                    bufs=2)
                                    for c in range(2):
                                        nc.tensor.matmul(
                                            ps_o,
                                            ctxTs[c][qb][:, col:col + 128],
                                            wo_sb[:, c,
                                                  512 * nb:512 * nb + 512],
                                            start=(c == 0), stop=(c == 1))
                                    dst = ost[:, 512 * nb:512 * nb + 512]
                                    if use_act and nb == 0:
                                        nc.scalar.copy(dst, ps_o)
                                    else:
                                        nc.vector.tensor_copy(dst, ps_o)
                                    if split:
                                        nc.sync.dma_start(
                                            out=outp[rows,
                                                     512 * nb:512 * nb + 512],
                                            in_=dst)
                                if not split:
                                    nc.sync.dma_start(out=outp[rows, :],
                                                      in_=ost)
                            return g
                        units.append(mk())
                    return units

                # ---------------- global emission ----------------
                def lab_units(units, pfx):
                    out = []
                    for i, u in enumerate(units):
                        def w(u=u, l=f"{pfx}.{i}"):
                            nc._set_label(l)
                            u()
                        out.append(w)
                    return out

                for u in lab_units(strip_units(0), "strip0"):
                    u()
                for qb in range(NQB):
                    bcu = lab_units(bc_units(qb), f"bc{qb}")
                    fill = []
                    if qb == 0:
                        fill += lab_units(strip_units(1), "strip1")
                    if qb == 1:
                        fill += lab_units(strip_units(2), "strip2")
                    if qb == 2:
                        fill += lab_units(strip_units(3), "strip3")
                    if qb == 3:
                        fill += lab_units(d_units(0), "d0")
                        fill += lab_units(d_units(1), "d1")
                        fill += lab_units(d_units(2), "d2")
                    for u in _interleave(fill, bcu):
                        u()
                for u in lab_units(d_units(3, use_act=True), "d3"):
                    u()

    nc.compile()
    return nc


_NC_CACHE = {}


def _get_nc(causal: bool):
    if causal not in _NC_CACHE:
        _NC_CACHE[causal] = _build_nc(causal)
    return _NC_CACHE[causal]


def _host_consts():
    p = np.zeros((128, 128), np.float32)
    idx = np.arange(0, 128, 2)
    p[idx, idx + 1] = -1.0
    p[idx + 1, idx] = 1.0
    psigT = np.ascontiguousarray(p.T)
    ident = np.eye(128, dtype=np.float32)
    m01n = np.where(np.arange(128)[None, :] >= np.arange(128)[:, None],
                    0.0, NEG).astype(np.float32)
    return psigT, ident, m01n


def _numpy_reference(hidden_states, cos, sin, attention_mask, Wq, Wk, Wv, Wo):
    """Generic-mask fallback, pure numpy port of the reference."""
    GROUPS = H // KVH

    def rope(x, c, s):
        c = c[:, None, :, :]
        s = s[:, None, :, :]
        x1, x2 = x[..., ::2], x[..., 1::2]
        xr = np.stack([x1 * c - x2 * s, x1 * s + x2 * c], axis=-1)
        return xr.reshape(x.shape)

    b, sq, d = hidden_states.shape
    q = (hidden_states @ Wq).reshape(b, sq, H, HD).transpose(0, 2, 1, 3)
    k = (hidden_states @ Wk).reshape(b, sq, KVH, HD).transpose(0, 2, 1, 3)
    v = (hidden_states @ Wv).reshape(b, sq, KVH, HD).transpose(0, 2, 1, 3)
    q = rope(q, cos, sin)
    k = rope(k, cos, sin)
    k = np.repeat(k, GROUPS, axis=1)
    v = np.repeat(v, GROUPS, axis=1)
    out = np.zeros((b, sq, d), np.float32)
    for bi in range(b):
        for hi in range(H):
            sc = (q[bi, hi] @ k[bi, hi].T) * SCALE + attention_mask[0, 0]
            sc = sc - sc.max(axis=-1, keepdims=True)
            e = np.exp(sc)
            pr = e / e.sum(axis=-1, keepdims=True)
            ctx = pr @ v[bi, hi]
            out[bi] += ctx @ Wo[hi * HD:(hi + 1) * HD]
    return out


def kernel(**inputs) -> np.ndarray:
    hs = np.asarray(inputs["hidden_states"], np.float32)
    cos = np.asarray(inputs["cos"], np.float32)
    sin = np.asarray(inputs["sin"], np.float32)
    mask = np.asarray(inputs["attention_mask"], np.float32)
    Wq = np.asarray(inputs["Wq"], np.float32)
    Wk = np.asarray(inputs["Wk"], np.float32)
    Wv = np.asarray(inputs["Wv"], np.float32)
    Wo = np.asarray(inputs["Wo"], np.float32)

    m = mask.reshape(S, S)
    tril = np.tril(np.ones((S, S), dtype=bool))
    causal_ref = np.where(tril, np.float32(0.0), np.float32(NEG))
    if np.array_equal(m, causal_ref):
        causal = True
    elif not m.any():
        causal = False
    else:
        return _numpy_reference(hs, cos, sin, mask, Wq, Wk, Wv, Wo)

    nc = _get_nc(causal)
    psigT, ident, m01 = _host_consts()
    chan_half = (np.arange(64) // 2)
    bf = ml_dtypes.bfloat16

    in_maps = []
    for core in range(8):
        b, t = core // TP, core % TP
        hT = np.ascontiguousarray(hs[b].T).astype(bf)
        hT3 = np.ascontiguousarray(
            hT.reshape(8, 128, S).transpose(1, 0, 2))
        cs_v = np.ascontiguousarray(cos[b].T[chan_half, :]).astype(bf)
        sn_v = np.ascontiguousarray(sin[b].T[chan_half, :]).astype(bf)
        wq_s = Wq[:, t * 256:(t + 1) * 256].astype(bf)
        wq4 = wq_s.reshape(8, 128, 2, 128).transpose(1, 0, 2, 3)
        wqA_h = np.ascontiguousarray(wq4[:, :, 0, :].reshape(128, 4, 256))
        wqB_h = np.ascontiguousarray(wq4[:, :, 1, :].reshape(128, 4, 256))
        wkv_s = np.concatenate([Wk[:, t * 64:(t + 1) * 64],
                                Wv[:, t * 64:(t + 1) * 64]], axis=1).astype(bf)
        wkv3 = np.ascontiguousarray(
            wkv_s.reshape(8, 128, 128).transpose(1, 0, 2)
            .reshape(128, 4, 256))
        wo_s = Wo[t * 256:(t + 1) * 256]
        # ctxT channel order per chunk: c0 = [h0|h2], c1 = [h1|h3]
        wo_p = np.concatenate([wo_s[0:64], wo_s[128:192],
                               wo_s[64:128], wo_s[192:256]], axis=0).astype(bf)
        wo3 = np.ascontiguousarray(wo_p.reshape(2, 128, D).transpose(1, 0, 2))
        in_maps.append({
            "hT3": hT3, "cs": cs_v, "sn": sn_v,
            "wqA": wqA_h, "wqB": wqB_h, "wkv3": wkv3, "wo3": wo3,
            "psigT": psigT.astype(bf), "ident": ident.astype(bf),
            "m01": m01.astype(bf),
            "onesc": np.ones((128, 64), bf),
        })

    res = run_bass_kernel_spmd(nc, in_maps, core_ids=list(range(8)))
    out = np.zeros((B, S, D), np.float32)
    for core in range(8):
        out[core // TP] += np.asarray(res.results[core]["out"],
                                      dtype=np.float32)
    return out
